# revision 1
# baseline (speedup 1.0000x reference)
"""BertSelfAttention (synthesizer mixture + symmetric ALiBi) Bass kernel for TRN2.

Data-parallel over batch: 8 cores x 2 batches each. One SPMD program.

Decomposition (per core, batches b=0,1; heads h=0..11):
  mw = softmax(mixture_weight)                          (host, 24 floats)
  aexp[h,j,i] = exp(mw1_h*synth_h[i,j] - slope_h*|i-j|) (host: content-INDEPENDENT
                - pure function of weights, like an ALiBi table)
  qT = (mw0_h/sqrt(64) * Wq) @ x.T                      (PE, transposed projection)
  kT = Wk @ x.T ; v = x @ Wv.T                          (PE)
  scT[j,i] = kT_h.T @ qT_h                              (PE, f32r)
  eT = exp(scT) * aexp[h]                               (ACT exp + DVE mul; no
       max-subtraction: scores empirically bounded in [-2.1, 2.2])
  ctx[i,:] = eT.T @ v_h ; rowsum[i] = eT.T @ 1          (PE)
  out[i, h*64:] = ctx * (1/rowsum)                      (DVE recip + scalar mul)

The softmax denominator is applied after the PV matmul, so probabilities are
never needed in the [i,j] orientation and no on-device transposes exist at all.
ALiBi banding: for high-slope heads, (jt,it) 128x128 tile pairs with
exp(-slope*dist) below ~1e-9 relative contribution are skipped entirely
(scores, exp, mul, pv, and the aexp DMA).
"""

from contextlib import ExitStack

import numpy as np

import concourse.bass as bass
import concourse.mybir as mybir
import concourse.tile as tile

F32 = mybir.dt.float32
F32R = mybir.dt.float32r  # fp32 storage; PE multiplies at reduced precision, 4x faster
BF16 = mybir.dt.bfloat16

H, S, D, DH = 12, 512, 768, 64
BPC = 2                # batches per core
T = BPC * S            # tokens per core
KT = D // 128          # contraction tiles over model dim
MT = T // 128          # token tiles per core
JT = S // 128          # key tiles per sequence


def _get_slopes(n):
    import math

    def pow2(n):
        start = 2 ** (-(2 ** (-(math.log2(n) - 3))))
        return [start * start**i for i in range(n)]

    if math.log2(n).is_integer():
        return pow2(n)
    cp2 = 2 ** math.floor(math.log2(n))
    return pow2(cp2) + _get_slopes(2 * cp2)[0::2][: n - cp2]


SLOPES = np.asarray(_get_slopes(H), np.float64)


def _band_dt(band_margin: float) -> list[int]:
    """Max |jt-it| (inclusive) per head; JT-1 means no banding.

    Tile pair (jt, it) has min element distance 128*|jt-it| - 127, so the
    pair is kept iff |jt-it| <= (L+127)//128 where L is the distance beyond
    which exp(-slope*d) is negligible relative to any kept element."""
    out = []
    for sl in SLOPES:
        L = int(np.ceil(band_margin / sl))
        out.append(min((L + 127) // 128, JT - 1))
    return out


def _r(ap):
    return ap.bitcast(F32R)


def _patch_tile_drain():
    """This walrus build rejects >1 sync-wait on one instruction; split the
    TileContext tail-drain's waits across single-wait drains."""
    from concourse.vector_clock import ScopedClock

    def _drain_and_barrier(self, tick_clock, wait_clock):
        nc = self.nc
        drain_inst = nc.sync.drain()
        wait_clock.add_sem_waits(
            drain_inst.ins, ScopedClock({None: tick_clock.global_clock})
        )
        waits = list(drain_inst.ins.sync_info.on_wait)
        if len(waits) > 1:
            drain_inst.ins.sync_info.on_wait = waits[:1]
            for w in waits[1:]:
                extra = nc.sync.drain()
                extra.ins.sync_info = mybir.SyncInfo(on_wait=[w], on_update=[])
        nc.all_engine_barrier()
        assert self.sems is not None
        popped = nc._tile_sem_poison_stack.pop()
        assert popped is self._sem_poison
        nc.clear_and_free_semaphores(list(self.sems.allocated().values()))
        nc.all_engine_barrier()

    tile.TileContext._drain_and_barrier = _drain_and_barrier


_patch_tile_drain()


def _split_multi_waits(nc):
    """This walrus build accepts at most one sync-wait per instruction; hoist
    extra waits onto single-wait NOPs emitted just before, on the same engine."""
    for fn in nc.m.functions:
        for bb in fn.blocks:
            out = []
            changed = False
            for ins in bb.instructions:
                si = ins.sync_info
                if si is not None and si.on_wait and len(si.on_wait) > 1:
                    waits = list(si.on_wait)
                    for i, w in enumerate(waits[:-1]):
                        nop = mybir.InstNoOp(
                            name=f"{ins.name}_w{i}",
                            engine=ins.engine,
                            sync_info=mybir.SyncInfo(on_wait=[w], on_update=[]),
                            bass_nofuse=True,
                        )
                        nc.register_instruction(nop, overwrite=True)
                        out.append(nop)
                    si.on_wait = waits[-1:]
                    changed = True
                out.append(ins)
            if changed:
                bb.instructions = out


def build_nc(probs_bf16: bool = True, band_margin: float = 14.0) -> bass.Bass:
    pdt = BF16 if probs_bf16 else F32
    band = _band_dt(band_margin)
    nc = bass.Bass("TRN2")
    xT = nc.dram_tensor("xT", [D, T], F32R, kind="ExternalInput").ap()
    wqT = nc.dram_tensor("wqT", [D, D], F32R, kind="ExternalInput").ap()
    wkT = nc.dram_tensor("wkT", [D, D], F32R, kind="ExternalInput").ap()
    wvT = nc.dram_tensor("wvT", [D, D], F32R, kind="ExternalInput").ap()
    aexp = nc.dram_tensor("aexp", [H, S, S], pdt, kind="ExternalInput").ap()
    out = nc.dram_tensor("out", [T, D], F32, kind="ExternalOutput").ap()

    with tile.TileContext(nc) as tc, ExitStack() as ctx:
        pers = ctx.enter_context(tc.tile_pool(name="pers", bufs=1))
        aexp_p = ctx.enter_context(tc.tile_pool(name="aexp_p", bufs=2))
        exp_p = ctx.enter_context(
            tc.tile_pool(name="exp_p", bufs=3 if probs_bf16 else 2)
        )
        r_p = ctx.enter_context(tc.tile_pool(name="r_p", bufs=2))
        psA = ctx.enter_context(tc.tile_pool(name="psA", bufs=2, space="PSUM"))
        psS = ctx.enter_context(tc.tile_pool(name="psS", bufs=3, space="PSUM"))
        psC = ctx.enter_context(tc.tile_pool(name="psC", bufs=2, space="PSUM"))
        psR = ctx.enter_context(tc.tile_pool(name="psR", bufs=1, space="PSUM"))

        qT_sb = pers.tile([128, KT, T], F32R, tag="qT")
        kT_sb = pers.tile([128, KT, T], F32R, tag="kT")
        v_sb = pers.tile([128, MT, D], pdt, tag="v")
        ones_sb = pers.tile([128, 1], pdt, tag="ones")
        out_sb = pers.tile([128, MT, D], F32, tag="outsb")
        xT_sb = pers.tile([128, KT, T], F32R, tag="xT")
        w_sbs = {}
        for name in ("q", "k", "v"):
            w_sbs[name] = pers.tile([128, KT, D], F32R, tag=f"w{name}", name=f"w{name}")

        nc.vector.memset(ones_sb, 1.0)
        for kt in range(KT):
            nc.sync.dma_start(out=xT_sb[:, kt, :], in_=xT[kt * 128 : (kt + 1) * 128, :])
        for name, w in (("v", wvT), ("q", wqT), ("k", wkT)):
            for kt in range(KT):
                nc.sync.dma_start(
                    out=w_sbs[name][:, kt, :], in_=w[kt * 128 : (kt + 1) * 128, :]
                )

        # ---- Interleaved projections + attention ----
        # Emit per feature-tile group gi: project q/k tile gi, some v chains,
        # then attention for heads 2gi, 2gi+1. Keeps ACT/DVE busy from ~1/6 of
        # phase A instead of waiting for all projections.
        def proj_qk(mt):
            for name, dst in (("q", qT_sb), ("k", kT_sb)):
                for nt in range(T // 512):
                    ps = psA.tile([128, 512], F32, tag="psA", name=f"psA_{name}{mt}{nt}")
                    for kt in range(KT):
                        nc.tensor.matmul(
                            ps,
                            lhsT=w_sbs[name][:, kt, mt * 128 : (mt + 1) * 128],
                            rhs=xT_sb[:, kt, nt * 512 : (nt + 1) * 512],
                            start=(kt == 0),
                            stop=(kt == KT - 1),
                        )
                    nc.scalar.copy(out=dst[:, mt, nt * 512 : (nt + 1) * 512], in_=ps)

        def proj_v(mt, half):
            n0, nw = (0, 512) if half == 0 else (512, 256)
            ps = psA.tile([128, 512], F32, tag="psA", name=f"psA_v{mt}{half}")
            for kt in range(KT):
                nc.tensor.matmul(
                    ps[:, :nw],
                    lhsT=xT_sb[:, kt, mt * 128 : (mt + 1) * 128],
                    rhs=w_sbs["v"][:, kt, n0 : n0 + nw],
                    start=(kt == 0),
                    stop=(kt == KT - 1),
                )
            nc.scalar.copy(out=v_sb[:, mt, n0 : n0 + nw], in_=ps[:, :nw])

        def attend(h):
            po, gi = (h % 2) * 64, h // 2  # qT/kT partition offset, feature tile
            dt_h = band[h]
            spans = []  # per jt: (i0, iw) kept column range
            for jt in range(JT):
                lo = max(0, jt - dt_h)
                hi = min(JT - 1, jt + dt_h)
                spans.append((lo * 128, (hi - lo + 1) * 128))
            ae = aexp_p.tile([128, JT, S], pdt, tag="ae", name=f"ae{h}")
            aeh = aexp[h].rearrange("(jt p) i -> p jt i", p=128)
            for jt in range(JT):
                i0, iw = spans[jt]
                nc.sync.dma_start(
                    out=ae[:, jt, i0 : i0 + iw], in_=aeh[:, jt, i0 : i0 + iw]
                )
            for b in range(BPC):
                t0 = b * S
                eT = exp_p.tile([128, JT, S], pdt, tag="eT", name=f"eT{h}{b}")
                for jt in range(JT):
                    i0, iw = spans[jt]
                    sc = psS.tile([128, S], F32, tag="sc", name=f"sc{h}{b}{jt}")
                    nc.tensor.matmul(
                        sc[:, i0 : i0 + iw],
                        lhsT=kT_sb[
                            po : po + DH, gi, t0 + jt * 128 : t0 + (jt + 1) * 128
                        ],
                        rhs=qT_sb[po : po + DH, gi, t0 + i0 : t0 + i0 + iw],
                        start=True,
                        stop=True,
                    )
                    nc.scalar.activation(
                        out=eT[:, jt, i0 : i0 + iw],
                        in_=sc[:, i0 : i0 + iw],
                        func=mybir.ActivationFunctionType.Exp,
                    )
                    nc.vector.tensor_mul(
                        out=eT[:, jt, i0 : i0 + iw],
                        in0=eT[:, jt, i0 : i0 + iw],
                        in1=ae[:, jt, i0 : i0 + iw],
                    )
                for it in range(JT):
                    jts = [jt for jt in range(JT) if abs(jt - it) <= dt_h]
                    cx = psC.tile([128, 64], F32, tag="cx", name=f"cx{h}{b}{it}")
                    rs = psR.tile([128, 1], F32, tag="rs", name=f"rs{h}{b}{it}")
                    for n, jt in enumerate(jts):
                        e_sl = eT[:, jt, it * 128 : (it + 1) * 128]
                        v_sl = v_sb[:, b * JT + jt, h * DH : (h + 1) * DH]
                        o_sl = ones_sb[:, :]
                        if pdt == F32:
                            e_sl, v_sl, o_sl = _r(e_sl), _r(v_sl), _r(o_sl)
                        nc.tensor.matmul(
                            cx,
                            lhsT=e_sl,
                            rhs=v_sl,
                            start=(n == 0),
                            stop=(n == len(jts) - 1),
                        )
                        nc.tensor.matmul(
                            rs,
                            lhsT=e_sl,
                            rhs=o_sl,
                            start=(n == 0),
                            stop=(n == len(jts) - 1),
                        )
                    r = r_p.tile([128, 1], F32, tag="r", name=f"r{h}{b}{it}")
                    nc.vector.reciprocal(out=r, in_=rs)
                    nc.vector.tensor_scalar_mul(
                        out=out_sb[:, b * JT + it, h * DH : (h + 1) * DH],
                        in0=cx,
                        scalar1=r,
                    )

        # v half-0 feeds heads 0-7's pv; emit those 8 chains first, then
        # interleave q/k tiles with attention; v half-1 woven in at gi 2-4.
        for mt in range(MT):
            proj_v(mt, 0)
        v1_sched = {2: [0, 1, 2], 3: [3, 4, 5], 4: [6, 7]}
        for gi in range(KT):
            proj_qk(gi)
            for mt in v1_sched.get(gi, []):
                proj_v(mt, 1)
            attend(2 * gi)
            attend(2 * gi + 1)

        for mt in range(MT):
            for c0 in (0, 192, 384, 576):
                nc.sync.dma_start(
                    out=out[mt * 128 : (mt + 1) * 128, c0 : c0 + 192],
                    in_=out_sb[:, mt, c0 : c0 + 192],
                )
    _split_multi_waits(nc)
    return nc


def host_prep(inputs: dict, probs_bf16: bool = True):
    """Returns (shared_inputs dict, per-core xT list)."""
    import ml_dtypes

    hs = np.ascontiguousarray(np.asarray(inputs["hidden_states"], np.float32))
    Wq = np.asarray(inputs["Wq"], np.float32)
    Wk = np.asarray(inputs["Wk"], np.float32)
    Wv = np.asarray(inputs["Wv"], np.float32)
    qfc = np.asarray(inputs["query_fc"], np.float32)
    kfc = np.asarray(inputs["key_fc"], np.float32)
    mwt = np.asarray(inputs["mixture_weight"], np.float32)[0, :, 0, 0, :]  # [H,2]

    e = np.exp(mwt - mwt.max(-1, keepdims=True))
    mw = e / e.sum(-1, keepdims=True)
    scale = np.repeat(mw[:, 0] / np.sqrt(DH), DH).astype(np.float32)

    wqT = np.ascontiguousarray((Wq * scale[:, None]).T)
    wkT = np.ascontiguousarray(Wk.T)
    wvT = np.ascontiguousarray(Wv.T)

    # content-independent bias table, transposed: [h, j, i]
    synthT = np.einsum("hik,hjk->hji", qfc, kfc).astype(np.float32)
    pos = np.arange(S)
    absd = np.abs(pos[None, :] - pos[:, None]).astype(np.float32)
    slopes = SLOPES.astype(np.float32)
    bias = mw[:, 1][:, None, None] * synthT - slopes[:, None, None] * absd[None]
    aexp = np.exp(bias)
    aexp = np.ascontiguousarray(
        aexp.astype(ml_dtypes.bfloat16 if probs_bf16 else np.float32)
    )

    shared = dict(wqT=wqT, wkT=wkT, wvT=wvT, aexp=aexp)
    n_cores = hs.shape[0] // BPC
    xTs = [
        np.ascontiguousarray(hs[c * BPC : (c + 1) * BPC].reshape(T, D).T)
        for c in range(n_cores)
    ]
    return shared, xTs


# ---------------------------------------------------------------------------
# Harness entry point: full (unsharded) inputs -> full output.
# Shards batch 16 -> 8 cores x 2, runs the SPMD Bass kernel, gathers.
# ---------------------------------------------------------------------------

N_CORES = 8
_NC_CACHE: dict = {}


def kernel(**inputs) -> np.ndarray:
    shared, xTs = host_prep(inputs, probs_bf16=True)
    if "nc" not in _NC_CACHE:
        _NC_CACHE["nc"] = build_nc(probs_bf16=True, band_margin=14.0)
    nc = _NC_CACHE["nc"]
    in_maps = [dict(shared, xT=xTs[c]) for c in range(N_CORES)]
    from concourse.bass_utils import run_bass_kernel_spmd

    res = run_bass_kernel_spmd(nc, in_maps, core_ids=list(range(N_CORES)))
    outs = [res.results[c]["out"].reshape(BPC, S, D) for c in range(N_CORES)]
    return np.concatenate(outs, axis=0).astype(np.float32)



# revision 39
# speedup vs baseline: 1.2883x; 1.2883x over previous
"""BertSelfAttention (synthesizer mixture + symmetric ALiBi) Bass kernel, TRN2.

Data-parallel over batch: 8 cores x 2 batches each. One SPMD program.

Per-core decomposition (b=0,1; heads h=0..11; token tiles of 128):
  host: mw = softmax(mixture_weight); fold mw0/sqrt(64) into Wq columns;
        ae[h,j,i] = exp(mw1*synth[h,i,j] - slope_h*|i-j|) (content-indep
        table, banded+packed, bf16);
        quantize Wq,Wk (per-column scale) and x (global scale) to fp8e4.
  PE:   qT,kT = W8.T @ x8 via fp8 DoubleRow matmuls (256-contraction,
        0.5 cyc/row); v = x @ WvT in bf16.
  DVE/ACT: psum->sbuf descale copies (per-partition 1/(ax*acol) scalars;
        ACT uses activation-Copy with scale).
  PE:   scT[j,i] = kT_h.T @ qT_h (bf16, 64-contraction), both batches into
        one 2-bank psum tile.
  ACT:  eT = exp(scT) for both batches in one instruction (no max
        subtraction: scores empirically bounded ~[-2.3, 2.3]).
  DVE:  eT *= ae (2x bf16 mode, ae broadcast across batches).
  PE:   [ctx | rowsum] = eT.T @ [v_h | 1] -- ones column interleaved in
        v layout (65 cols/head) so rowsum rides the same accumulation.
  ACT:  r = 1/rowsum (activation Reciprocal).
  DVE:  out = ctx * r, fused per (h,b) over all 4 row tiles.

ALiBi banding: (jt,it) tile pairs with negligible contribution skipped
(scores, exp, mul, pv, and the ae columns). attend() is split into a
scores phase and a per-batch PV phase so PE's in-order stream never
blocks on V projections; V-batch-0 PV only needs v rows 0..511.
"""

from contextlib import ExitStack

import numpy as np

import concourse.bass as bass
import concourse.mybir as mybir
import concourse.tile as tile

F32 = mybir.dt.float32
BF16 = mybir.dt.bfloat16
FP8 = mybir.dt.float8e4

H, S, D, DH = 12, 512, 768, 64
BPC = 2                # batches per core
T = BPC * S            # tokens per core
KT = D // 128          # bf16 contraction tiles over model dim
KP = D // 256          # fp8 DoubleRow contraction pair-tiles
MT = T // 128          # token tiles per core
JT = S // 128          # key tiles per sequence
VW = H * 65            # v columns incl. interleaved ones column per head
FP8_MAX = 192.0        # target max for float8_e4m3 (type max 240)

# max |jt-it| kept per head (dropped tile-pairs contribute <~1e-3 of rowsum)
BAND = [1, 1, 1, 1, 2, 3, 3, 3, 1, 1, 1, 1]
# fine alibi reach per head (exp/scores shrink to these spans; ae-table
# zeros + the tile-span multiply zero-fill the remainder)
FINE_L = [32, 32, 64, 128, 256, 99999, 99999, 99999, 32, 32, 64, 96]


def _get_slopes(n):
    import math

    def pow2(n):
        start = 2 ** (-(2 ** (-(math.log2(n) - 3))))
        return [start * start**i for i in range(n)]

    if math.log2(n).is_integer():
        return pow2(n)
    cp2 = 2 ** math.floor(math.log2(n))
    return pow2(cp2) + _get_slopes(2 * cp2)[0::2][: n - cp2]


SLOPES = np.asarray(_get_slopes(H), np.float64)


def _spans(dt_h):
    """Per jt: (i0, iw) kept column range (PV-tile granular)."""
    out = []
    for jt in range(JT):
        lo = max(0, jt - dt_h)
        hi = min(JT - 1, jt + dt_h)
        out.append((lo * 128, (hi - lo + 1) * 128))
    return out


def _fine_spans(dt_h, L):
    """Per jt: (i0, iw) 64-granular span [jt*128-L, (jt+1)*128+L) clipped to
    the tile span. Columns outside carry ae=0, so eT there is zero-filled by
    the multiply."""
    out = []
    for jt, (t0, tw) in enumerate(_spans(dt_h)):
        lo = max(t0, jt * 128 - L)
        hi = min(t0 + tw, (jt + 1) * 128 + L)
        out.append((lo, hi - lo))
    return out


def _ae_layout(band):
    """Per-head per-jt column offsets into the packed ae table."""
    offs, total = [], 0
    for h in range(H):
        sp = _spans(band[h])
        jt_off = []
        for jt in range(JT):
            jt_off.append(total)
            total += sp[jt][1]
        offs.append(jt_off)
    return offs, total


def _patch_tile_drain():
    """This walrus build rejects >1 sync-wait on one instruction; split the
    TileContext tail-drain's waits across single-wait drains."""
    from concourse.vector_clock import ScopedClock

    def _drain_and_barrier(self, tick_clock, wait_clock):
        nc = self.nc
        drain_inst = nc.sync.drain()
        wait_clock.add_sem_waits(
            drain_inst.ins, ScopedClock({None: tick_clock.global_clock})
        )
        waits = list(drain_inst.ins.sync_info.on_wait)
        if len(waits) > 1:
            drain_inst.ins.sync_info.on_wait = waits[:1]
            for w in waits[1:]:
                extra = nc.sync.drain()
                extra.ins.sync_info = mybir.SyncInfo(on_wait=[w], on_update=[])
        nc.all_engine_barrier()
        assert self.sems is not None
        popped = nc._tile_sem_poison_stack.pop()
        assert popped is self._sem_poison
        nc.clear_and_free_semaphores(list(self.sems.allocated().values()))
        nc.all_engine_barrier()

    tile.TileContext._drain_and_barrier = _drain_and_barrier


_patch_tile_drain()


def _split_multi_waits(nc):
    """This walrus build accepts at most one sync-wait per instruction; hoist
    extra waits onto single-wait NOPs emitted just before, on the same engine."""
    for fn in nc.m.functions:
        for bb in fn.blocks:
            out = []
            changed = False
            for ins in bb.instructions:
                si = ins.sync_info
                if si is not None and si.on_wait and len(si.on_wait) > 1:
                    waits = list(si.on_wait)
                    for i, w in enumerate(waits[:-1]):
                        nop = mybir.InstNoOp(
                            name=f"{ins.name}_w{i}",
                            engine=ins.engine,
                            sync_info=mybir.SyncInfo(on_wait=[w], on_update=[]),
                            bass_nofuse=True,
                        )
                        nc.register_instruction(nop, overwrite=True)
                        out.append(nop)
                    si.on_wait = waits[-1:]
                    changed = True
                out.append(ins)
            if changed:
                bb.instructions = out


def build_nc(band=None) -> bass.Bass:
    band = band or BAND
    ae_offs, ae_total = _ae_layout(band)
    nc = bass.Bass("TRN2")
    x8 = nc.dram_tensor("x8", [D, T], FP8, kind="ExternalInput").ap()
    x8l = nc.dram_tensor("x8l", [D, T], FP8, kind="ExternalInput").ap()
    wq8 = nc.dram_tensor("wq8", [D, D], FP8, kind="ExternalInput").ap()
    wq8l = nc.dram_tensor("wq8l", [D, D], FP8, kind="ExternalInput").ap()
    wk8 = nc.dram_tensor("wk8", [D, D], FP8, kind="ExternalInput").ap()
    wk8l = nc.dram_tensor("wk8l", [D, D], FP8, kind="ExternalInput").ap()
    wv8 = nc.dram_tensor("wv8", [D, D], FP8, kind="ExternalInput").ap()
    wv8l = nc.dram_tensor("wv8l", [D, D], FP8, kind="ExternalInput").ap()
    dsc = nc.dram_tensor("dsc", [128, 2 * KT + 1], F32, kind="ExternalInput").ap()
    ae = nc.dram_tensor("ae", [128, ae_total], BF16, kind="ExternalInput").ap()
    out = nc.dram_tensor("out", [T, D], BF16, kind="ExternalOutput").ap()

    with tile.TileContext(nc) as tc, ExitStack() as ctx:
        pers = ctx.enter_context(tc.tile_pool(name="pers", bufs=1))
        aexp_p = ctx.enter_context(tc.tile_pool(name="aexp_p", bufs=6))
        exp_p = ctx.enter_context(tc.tile_pool(name="exp_p", bufs=8))
        r_p = ctx.enter_context(tc.tile_pool(name="r_p", bufs=2))
        psA = ctx.enter_context(tc.tile_pool(name="psA", bufs=2, space="PSUM"))
        psS = ctx.enter_context(tc.tile_pool(name="psS", bufs=2, space="PSUM"))
        psC = ctx.enter_context(tc.tile_pool(name="psC", bufs=2, space="PSUM"))

        x8_sb = pers.tile([128, KP, 2, T], FP8, tag="x8sb")
        x8l_sb = pers.tile([128, KP, 2, T], FP8, tag="x8lsb")
        wq8_sb = pers.tile([128, KP, 2, D], FP8, tag="wq8sb")
        wq8l_sb = pers.tile([128, KP, 2, D], FP8, tag="wq8lsb")
        wk8_sb = pers.tile([128, KP, 2, D], FP8, tag="wk8sb")
        wk8l_sb = pers.tile([128, KP, 2, D], FP8, tag="wk8lsb")
        wv8_sb = pers.tile([128, KP, 2, D], FP8, tag="wv8sb")
        wv8l_sb = pers.tile([128, KP, 2, D], FP8, tag="wv8lsb")
        dsc_sb = pers.tile([128, 2 * KT + 1], F32, tag="dscsb")
        kq_sb = pers.tile([128, 2, KT, T], BF16, tag="kqsb")
        v_sb = pers.tile([128, MT, VW], BF16, tag="vsb")
        out_sb = pers.tile([128, MT, D], BF16, tag="outsb")

        # ---- input DMAs, ordered by first use; first-needed tiles split out ----
        nc.sync.dma_start(out=dsc_sb, in_=dsc)

        def pack8(t):
            return t.rearrange("(kp half p) t -> p kp half t", kp=KP, half=2)

        nc.sync.dma_start(out=wq8_sb[:, 0], in_=pack8(wq8)[:, 0])
        nc.sync.dma_start(out=x8_sb[:, 0], in_=pack8(x8)[:, 0])
        nc.sync.dma_start(out=wk8_sb[:, 0], in_=pack8(wk8)[:, 0])
        nc.sync.dma_start(out=wq8_sb[:, 1:], in_=pack8(wq8)[:, 1:])
        nc.sync.dma_start(out=x8_sb[:, 1:], in_=pack8(x8)[:, 1:])
        nc.sync.dma_start(out=wk8_sb[:, 1:], in_=pack8(wk8)[:, 1:])
        nc.sync.dma_start(out=x8l_sb, in_=pack8(x8l))
        nc.sync.dma_start(out=wq8l_sb, in_=pack8(wq8l))
        nc.sync.dma_start(out=wk8l_sb, in_=pack8(wk8l))
        nc.sync.dma_start(out=wv8_sb, in_=pack8(wv8))
        nc.sync.dma_start(out=wv8l_sb, in_=pack8(wv8l))

        ae_tiles = {}

        def ae_load(h):
            w = ae_offs[h][JT - 1] + _spans(band[h])[JT - 1][1] - ae_offs[h][0]
            t = aexp_p.tile([128, w], BF16, tag="ae", name=f"ae{h}")
            nc.sync.dma_start(out=t, in_=ae[:, ae_offs[h][0] : ae_offs[h][0] + w])
            ae_tiles[h] = t

        for h in range(6):
            ae_load(h)

        # ones column per head for the fused rowsum
        v65 = v_sb.rearrange("p m (h c) -> p m h c", c=65)
        nc.vector.memset(v65[:, :, :, 64:65], 1.0)

        # ---- projections ----
        def proj_qk(pi, gi):
            """pi: 0=q (DVE descale copy), 1=k (ACT descale copy).
            3 fp8 rails: x8*w8 + x8*w8l + x8l*w8 (score noise ~0.1%)."""
            w_sb, wl_sb = (wq8_sb, wq8l_sb) if pi == 0 else (wk8_sb, wk8l_sb)
            rails = ((x8_sb, w_sb), (x8_sb, wl_sb), (x8l_sb, w_sb))
            dc = gi if pi == 0 else KT + gi
            for cc in range(2):
                pA = psA.tile([128, 2, 256], F32, tag="psA", name=f"pA{pi}{gi}{cc}")
                for ci in range(2):
                    c = 2 * cc + ci
                    n = 0
                    for xs, ws in rails:
                        for kp in range(KP):
                            nc.tensor.matmul(
                                pA[:, ci, :],
                                lhsT=ws[:, kp, :, gi * 128 : (gi + 1) * 128],
                                rhs=xs[:, kp, :, c * 256 : (c + 1) * 256],
                                start=(n == 0),
                                stop=(n == 3 * KP - 1),
                                perf_mode=mybir.MatmulPerfMode.DoubleRow,
                            )
                            n += 1
                dst = kq_sb[:, pi, gi, cc * 512 : (cc + 1) * 512]
                if pi == 0:
                    nc.vector.tensor_scalar_mul(
                        out=dst, in0=pA, scalar1=dsc_sb[:, dc : dc + 1]
                    )
                else:
                    nc.scalar.activation(
                        out=dst,
                        in_=pA,
                        func=mybir.ActivationFunctionType.Copy,
                        scale=dsc_sb[:, dc : dc + 1],
                    )

        def proj_v(mt, half):
            # dual-rail fp8 DoubleRow: v = x8*wv8 + x8*wv8l + x8l*wv8
            # (all rails at scale ax*aw; one uniform descale at the copy)
            c0, cw = (0, 512) if half == 0 else (512, 256)
            pV = psA.tile([128, 512], F32, tag="psA", name=f"pV{mt}{half}")
            rails = ((x8_sb, wv8_sb), (x8_sb, wv8l_sb), (x8l_sb, wv8_sb))
            for ci in range(cw // 256):
                c = c0 + ci * 256
                n = 0
                for xs, ws in rails:
                    for kp in range(KP):
                        nc.tensor.matmul(
                            pV[:, ci * 256 : ci * 256 + 256]
                            if half == 0
                            else pV[:, 0:256],
                            lhsT=xs[:, kp, :, mt * 128 : (mt + 1) * 128],
                            rhs=ws[:, kp, :, c : c + 256],
                            start=(n == 0),
                            stop=(n == 3 * KP - 1),
                            perf_mode=mybir.MatmulPerfMode.DoubleRow,
                        )
                        n += 1
            dv = dsc_sb[:, 2 * KT : 2 * KT + 1]
            if half == 0:
                nc.vector.tensor_scalar_mul(
                    out=v65[:, mt, 0:8, 0:64], in0=pV[:, :cw], scalar1=dv
                )
            else:
                nc.scalar.activation(
                    out=v65[:, mt, 8:12, 0:64],
                    in_=pV[:, 0:cw],
                    func=mybir.ActivationFunctionType.Copy,
                    scale=dv,
                )

        # ---- attention: scores phase ----
        eT_tiles = {}

        def att_sc(h, jts):
            gi, po = h // 2, (h % 2) * 64
            spans = _spans(band[h])
            # first pool rotation (h<6): eT buffers are fresh SBUF, so exp must
            # define the full tile span; afterwards stale-but-finite contents
            # let the ae=0 multiply zero-fill the fine-span gaps.
            fine = spans if h < 6 else _fine_spans(band[h], FINE_L[h])
            if h not in eT_tiles:
                eT_tiles[h] = exp_p.tile([128, JT, 2, S], BF16, tag="eT", name=f"eT{h}")
            eT = eT_tiles[h]
            for jt in jts:
                f0, fw = fine[jt]
                sc = psS.tile([128, 2, S], F32, tag="sc", name=f"sc{h}{jt}")
                for b in range(BPC):
                    t0 = b * S
                    nc.tensor.matmul(
                        sc[:, b, f0 : f0 + fw],
                        lhsT=kq_sb[
                            po : po + DH, 1, gi, t0 + jt * 128 : t0 + (jt + 1) * 128
                        ],
                        rhs=kq_sb[po : po + DH, 0, gi, t0 + f0 : t0 + f0 + fw],
                        start=True,
                        stop=True,
                    )
                nc.scalar.activation(
                    out=eT[:, jt, :, f0 : f0 + fw],
                    in_=sc[:, :, f0 : f0 + fw],
                    func=mybir.ActivationFunctionType.Exp,
                )

        def att_mul(h):
            spans = _spans(band[h])
            if 6 <= h + 2 < H:
                ae_load(h + 2)
            aeh = ae_tiles[h]
            ae0 = ae_offs[h][0]
            eT = eT_tiles[h]
            for jt in range(JT):
                i0, iw = spans[jt]
                off = ae_offs[h][jt] - ae0
                nc.vector.tensor_mul(
                    out=eT[:, jt, :, i0 : i0 + iw],
                    in0=eT[:, jt, :, i0 : i0 + iw],
                    in1=aeh[:, off : off + iw].unsqueeze(1).broadcast_to((128, 2, iw)),
                )

        # ---- attention: per-batch PV + normalize phase ----
        def att_pv(h, b):
            dt_h = band[h]
            eT = eT_tiles[h]
            pC = psC.tile([128, JT, 128], F32, tag="pC", name=f"pC{h}{b}")
            for it in range(JT):
                jts = [jt for jt in range(JT) if abs(jt - it) <= dt_h]
                for n, jt in enumerate(jts):
                    nc.tensor.matmul(
                        pC[:, it, 0:65],
                        lhsT=eT[:, jt, b, it * 128 : (it + 1) * 128],
                        rhs=v_sb[:, b * JT + jt, h * 65 : h * 65 + 65],
                        start=(n == 0),
                        stop=(n == len(jts) - 1),
                    )
            r = r_p.tile([128, JT], F32, tag="r", name=f"r{h}{b}")
            nc.vector.reciprocal(out=r, in_=pC[:, :, 64:65])
            nc.vector.tensor_mul(
                out=out_sb[:, b * JT : (b + 1) * JT, h * DH : (h + 1) * DH],
                in0=pC[:, :, 0:64],
                in1=r.unsqueeze(2).broadcast_to((128, JT, DH)),
            )

        def out_chunk(c0, cw):
            # column chunk across all token tiles: row (mt,p), cols c0:c0+cw
            nc.sync.dma_start(
                out=out.rearrange("(mt p) d -> p mt d", p=128)[:, :, c0 : c0 + cw],
                in_=out_sb[:, :, c0 : c0 + cw],
            )

        # ---- schedule: strict rounds. Per round r: project ft-tile r (its
        # psum-freeing copies land at the HEAD of the DVE/ACT queues, before
        # this round's muls), score heads 2r/2r+1 with the two heads' jt
        # tiles interleaved (hides exp latency behind the other head's
        # matmuls), multiply, then PV heads from two rounds back. ----
        def sc_pair(a):
            att_sc(a, (0, 1))
            att_sc(a + 1, (0, 1))
            att_sc(a, (2, 3))
            att_sc(a + 1, (2, 3))

        # r0
        proj_qk(0, 0)
        proj_qk(1, 0)
        sc_pair(0)
        att_mul(0)
        att_mul(1)
        # r1
        proj_qk(0, 1)
        proj_qk(1, 1)
        sc_pair(2)
        for mt in range(4):
            proj_v(mt, 0)
        att_mul(2)
        att_mul(3)
        # r2
        proj_qk(0, 2)
        proj_qk(1, 2)
        sc_pair(4)
        for mt in range(4, MT):
            proj_v(mt, 0)
        att_mul(4)
        att_mul(5)
        att_pv(0, 0)
        att_pv(1, 0)
        att_pv(0, 1)
        att_pv(1, 1)
        # r3
        proj_qk(0, 3)
        proj_qk(1, 3)
        sc_pair(6)
        att_mul(6)
        att_mul(7)
        att_pv(2, 0)
        att_pv(2, 1)
        att_pv(3, 0)
        att_pv(3, 1)
        out_chunk(0, 256)
        # r4
        proj_qk(0, 4)
        proj_qk(1, 4)
        sc_pair(8)
        for mt in range(4):
            proj_v(mt, 1)
        att_mul(8)
        att_mul(9)
        att_pv(4, 0)
        att_pv(4, 1)
        att_pv(5, 0)
        att_pv(5, 1)
        # r5
        proj_qk(0, 5)
        proj_qk(1, 5)
        sc_pair(10)
        for mt in range(4, MT):
            proj_v(mt, 1)
        att_mul(10)
        att_mul(11)
        att_pv(6, 0)
        att_pv(6, 1)
        att_pv(7, 0)
        att_pv(7, 1)
        out_chunk(256, 256)
        att_pv(8, 0)
        att_pv(9, 0)
        # r6
        att_pv(8, 1)
        att_pv(9, 1)
        att_pv(10, 0)
        att_pv(10, 1)
        att_pv(11, 0)
        att_pv(11, 1)
        out_chunk(512, 256)
    _split_multi_waits(nc)
    return nc


def host_prep(inputs: dict, band=None):
    """Returns (shared input dict, per-core dict list)."""
    import ml_dtypes

    band = band or BAND
    ae_offs, ae_total = _ae_layout(band)

    hs = np.ascontiguousarray(np.asarray(inputs["hidden_states"], np.float32))
    Wq = np.asarray(inputs["Wq"], np.float32)
    Wk = np.asarray(inputs["Wk"], np.float32)
    Wv = np.asarray(inputs["Wv"], np.float32)
    qfc = np.asarray(inputs["query_fc"], np.float32)
    kfc = np.asarray(inputs["key_fc"], np.float32)
    mwt = np.asarray(inputs["mixture_weight"], np.float32)[0, :, 0, 0, :]  # [H,2]

    e = np.exp(mwt - mwt.max(-1, keepdims=True))
    mw = e / e.sum(-1, keepdims=True)
    qscale = np.repeat(mw[:, 0] / np.sqrt(DH), DH).astype(np.float32)  # per out col

    ax = FP8_MAX / max(np.abs(hs).max(), 1e-30)

    def quant_w(wT):  # wT [in_feat, out_feat] -> (fp8 array, descale per col)
        amax = np.abs(wT).max(axis=0)
        ac = FP8_MAX / np.maximum(amax, 1e-30)
        w8 = (wT * ac[None, :]).astype(ml_dtypes.float8_e4m3)
        return w8, (1.0 / (ax * ac)).astype(np.float32)

    def quant_w_rails(wT):
        w8, d = quant_w(wT)
        ac = FP8_MAX / np.maximum(np.abs(wT).max(axis=0), 1e-30)
        w8l = ((wT * ac[None, :]) - w8.astype(np.float32)).astype(
            ml_dtypes.float8_e4m3
        )
        return w8, w8l, d

    wq8, wq8l, dq = quant_w_rails(Wq.T * qscale[None, :])
    wk8, wk8l, dk = quant_w_rails(Wk.T)

    # dual-rail fp8 V weights at one global scale
    wvT = Wv.T.astype(np.float32)
    aw = FP8_MAX / max(np.abs(wvT).max(), 1e-30)
    wv8 = (wvT * aw).astype(ml_dtypes.float8_e4m3)
    wv8l = ((wvT - wv8.astype(np.float32) / aw) * aw).astype(ml_dtypes.float8_e4m3)
    dv = np.full((128, 1), 1.0 / (ax * aw), np.float32)

    dsc = np.concatenate(
        [dq.reshape(KT, 128).T, dk.reshape(KT, 128).T, dv], axis=1
    )  # [128, 2*KT+1]
    dsc = np.ascontiguousarray(dsc)

    # packed banded bias table: ae[h][p, off+e] = exp(mw1*synth^T + alibi)
    synthT = np.einsum("hik,hjk->hji", qfc, kfc).astype(np.float32)
    pos = np.arange(S)
    absd = np.abs(pos[None, :] - pos[:, None]).astype(np.float32)
    slopes = SLOPES.astype(np.float32)
    bias = mw[:, 1][:, None, None] * synthT - slopes[:, None, None] * absd[None]
    aexp = np.exp(bias)  # [H, S(j), S(i)]
    ae_pack = np.zeros((128, ae_total), np.float32)
    for h in range(H):
        sp = _spans(band[h])
        fine = _fine_spans(band[h], FINE_L[h])
        for jt in range(JT):
            i0, iw = sp[jt]
            blk = aexp[h, jt * 128 : (jt + 1) * 128, i0 : i0 + iw].copy()
            f0, fw = fine[jt]
            blk[:, : f0 - i0] = 0.0
            blk[:, f0 - i0 + fw :] = 0.0
            ae_pack[:, ae_offs[h][jt] : ae_offs[h][jt] + iw] = blk
    ae_pack = np.ascontiguousarray(ae_pack.astype(ml_dtypes.bfloat16))

    shared = dict(
        wq8=wq8, wq8l=wq8l, wk8=wk8, wk8l=wk8l, wv8=wv8, wv8l=wv8l, dsc=dsc, ae=ae_pack
    )
    n_cores = hs.shape[0] // BPC
    per_core = []
    for c in range(n_cores):
        xT = np.ascontiguousarray(hs[c * BPC : (c + 1) * BPC].reshape(T, D).T)
        x8 = (xT * ax).astype(ml_dtypes.float8_e4m3)
        x8l = ((xT - x8.astype(np.float32) / ax) * ax).astype(ml_dtypes.float8_e4m3)
        per_core.append(
            dict(x8=np.ascontiguousarray(x8), x8l=np.ascontiguousarray(x8l))
        )
    return shared, per_core


# ---------------------------------------------------------------------------
# Harness entry point: full (unsharded) inputs -> full output.
# Shards batch 16 -> 8 cores x 2, runs the SPMD Bass kernel, gathers.
# ---------------------------------------------------------------------------

N_CORES = 8
_NC_CACHE: dict = {}


def kernel(**inputs) -> np.ndarray:
    shared, per_core = host_prep(inputs)
    if "nc" not in _NC_CACHE:
        _NC_CACHE["nc"] = build_nc()
    nc = _NC_CACHE["nc"]
    in_maps = [dict(shared, **per_core[c]) for c in range(N_CORES)]
    from concourse.bass_utils import run_bass_kernel_spmd

    res = run_bass_kernel_spmd(nc, in_maps, core_ids=list(range(N_CORES)))
    outs = [
        res.results[c]["out"].astype(np.float32).reshape(BPC, S, D)
        for c in range(N_CORES)
    ]
    return np.concatenate(outs, axis=0)


# revision 45
# speedup vs baseline: 1.3742x; 1.0667x over previous
"""BertSelfAttention (synthesizer mixture + symmetric ALiBi) Bass kernel, TRN2.

Data-parallel over batch: 8 cores x 2 batches each. One SPMD program.

Per-core decomposition (b=0,1; heads h=0..11; token tiles of 128):
  host: mw = softmax(mixture_weight); fold mw0/sqrt(64) into Wq columns;
        ae[h,j,i] = exp(mw1*synth[h,i,j] - slope_h*|i-j|) (content-indep
        table, banded+packed, bf16);
        quantize Wq,Wk (per-column scale) and x (global scale) to fp8e4.
  PE:   qT,kT = W8.T @ x8 via fp8 DoubleRow matmuls (256-contraction,
        0.5 cyc/row); v = x @ WvT in bf16.
  DVE/ACT: psum->sbuf descale copies (per-partition 1/(ax*acol) scalars;
        ACT uses activation-Copy with scale).
  PE:   scT[j,i] = kT_h.T @ qT_h (bf16, 64-contraction), both batches into
        one 2-bank psum tile.
  ACT:  eT = exp(scT) for both batches in one instruction (no max
        subtraction: scores empirically bounded ~[-2.3, 2.3]).
  DVE:  eT *= ae (2x bf16 mode, ae broadcast across batches).
  PE:   [ctx | rowsum] = eT.T @ [v_h | 1] -- ones column interleaved in
        v layout (65 cols/head) so rowsum rides the same accumulation.
  ACT:  r = 1/rowsum (activation Reciprocal).
  DVE:  out = ctx * r, fused per (h,b) over all 4 row tiles.

ALiBi banding: (jt,it) tile pairs with negligible contribution skipped
(scores, exp, mul, pv, and the ae columns). attend() is split into a
scores phase and a per-batch PV phase so PE's in-order stream never
blocks on V projections; V-batch-0 PV only needs v rows 0..511.
"""

from contextlib import ExitStack

import numpy as np

import concourse.bass as bass
import concourse.mybir as mybir
import concourse.tile as tile

F32 = mybir.dt.float32
BF16 = mybir.dt.bfloat16
FP8 = mybir.dt.float8e4

H, S, D, DH = 12, 512, 768, 64
BPC = 2                # batches per core
T = BPC * S            # tokens per core
KT = D // 128          # bf16 contraction tiles over model dim
KP = D // 256          # fp8 DoubleRow contraction pair-tiles
MT = T // 128          # token tiles per core
JT = S // 128          # key tiles per sequence
VW = H * 65            # v columns incl. interleaved ones column per head
FP8_MAX = 192.0        # target max for float8_e4m3 (type max 240)

# max |jt-it| kept per head (dropped tile-pairs contribute <~1e-3 of rowsum)
BAND = [1, 1, 1, 1, 2, 3, 3, 3, 1, 1, 1, 1]
# fine alibi reach per head (exp/scores shrink to these spans; ae-table
# zeros + the tile-span multiply zero-fill the remainder)
FINE_L = [32, 32, 64, 128, 256, 99999, 99999, 99999, 32, 32, 64, 96]


def _get_slopes(n):
    import math

    def pow2(n):
        start = 2 ** (-(2 ** (-(math.log2(n) - 3))))
        return [start * start**i for i in range(n)]

    if math.log2(n).is_integer():
        return pow2(n)
    cp2 = 2 ** math.floor(math.log2(n))
    return pow2(cp2) + _get_slopes(2 * cp2)[0::2][: n - cp2]


SLOPES = np.asarray(_get_slopes(H), np.float64)


def _spans(dt_h):
    """Per jt: (i0, iw) kept column range (PV-tile granular)."""
    out = []
    for jt in range(JT):
        lo = max(0, jt - dt_h)
        hi = min(JT - 1, jt + dt_h)
        out.append((lo * 128, (hi - lo + 1) * 128))
    return out


def _fine_spans(dt_h, L):
    """Per jt: (i0, iw) 64-granular span [jt*128-L, (jt+1)*128+L) clipped to
    the tile span. Columns outside carry ae=0, so eT there is zero-filled by
    the multiply."""
    out = []
    for jt, (t0, tw) in enumerate(_spans(dt_h)):
        lo = max(t0, jt * 128 - L)
        hi = min(t0 + tw, (jt + 1) * 128 + L)
        out.append((lo, hi - lo))
    return out


def _ae_layout(band):
    """Per-head per-jt column offsets into the packed ae table."""
    offs, total = [], 0
    for h in range(H):
        sp = _spans(band[h])
        jt_off = []
        for jt in range(JT):
            jt_off.append(total)
            total += sp[jt][1]
        offs.append(jt_off)
    return offs, total


def _patch_tile_drain():
    """This walrus build rejects >1 sync-wait on one instruction; split the
    TileContext tail-drain's waits across single-wait drains."""
    from concourse.vector_clock import ScopedClock

    def _drain_and_barrier(self, tick_clock, wait_clock):
        nc = self.nc
        drain_inst = nc.sync.drain()
        wait_clock.add_sem_waits(
            drain_inst.ins, ScopedClock({None: tick_clock.global_clock})
        )
        waits = list(drain_inst.ins.sync_info.on_wait)
        if len(waits) > 1:
            drain_inst.ins.sync_info.on_wait = waits[:1]
            for w in waits[1:]:
                extra = nc.sync.drain()
                extra.ins.sync_info = mybir.SyncInfo(on_wait=[w], on_update=[])
        nc.all_engine_barrier()
        assert self.sems is not None
        popped = nc._tile_sem_poison_stack.pop()
        assert popped is self._sem_poison
        nc.clear_and_free_semaphores(list(self.sems.allocated().values()))
        nc.all_engine_barrier()

    tile.TileContext._drain_and_barrier = _drain_and_barrier


_patch_tile_drain()


def _split_multi_waits(nc):
    """This walrus build accepts at most one sync-wait per instruction; hoist
    extra waits onto single-wait NOPs emitted just before, on the same engine."""
    for fn in nc.m.functions:
        for bb in fn.blocks:
            out = []
            changed = False
            for ins in bb.instructions:
                si = ins.sync_info
                if si is not None and si.on_wait and len(si.on_wait) > 1:
                    waits = list(si.on_wait)
                    for i, w in enumerate(waits[:-1]):
                        nop = mybir.InstNoOp(
                            name=f"{ins.name}_w{i}",
                            engine=ins.engine,
                            sync_info=mybir.SyncInfo(on_wait=[w], on_update=[]),
                            bass_nofuse=True,
                        )
                        nc.register_instruction(nop, overwrite=True)
                        out.append(nop)
                    si.on_wait = waits[-1:]
                    changed = True
                out.append(ins)
            if changed:
                bb.instructions = out


def build_nc(band=None) -> bass.Bass:
    band = band or BAND
    ae_offs, ae_total = _ae_layout(band)
    nc = bass.Bass("TRN2")
    x8 = nc.dram_tensor("x8", [D, T], FP8, kind="ExternalInput").ap()
    x8l = nc.dram_tensor("x8l", [D, T], FP8, kind="ExternalInput").ap()
    wq8 = nc.dram_tensor("wq8", [D, D], FP8, kind="ExternalInput").ap()
    wq8l = nc.dram_tensor("wq8l", [D, D], FP8, kind="ExternalInput").ap()
    wk8 = nc.dram_tensor("wk8", [D, D], FP8, kind="ExternalInput").ap()
    wk8l = nc.dram_tensor("wk8l", [D, D], FP8, kind="ExternalInput").ap()
    wv8 = nc.dram_tensor("wv8", [D, D], FP8, kind="ExternalInput").ap()
    wv8l = nc.dram_tensor("wv8l", [D, D], FP8, kind="ExternalInput").ap()
    dsc = nc.dram_tensor("dsc", [128, 2 * KT + 1], F32, kind="ExternalInput").ap()
    ae = nc.dram_tensor("ae", [128, ae_total], BF16, kind="ExternalInput").ap()
    out = nc.dram_tensor("out", [T, D], BF16, kind="ExternalOutput").ap()

    with tile.TileContext(nc) as tc, ExitStack() as ctx:
        pers = ctx.enter_context(tc.tile_pool(name="pers", bufs=1))
        aexp_p = ctx.enter_context(tc.tile_pool(name="aexp_p", bufs=6))
        exp_p = ctx.enter_context(tc.tile_pool(name="exp_p", bufs=8))
        r_p = ctx.enter_context(tc.tile_pool(name="r_p", bufs=2))
        psA = ctx.enter_context(tc.tile_pool(name="psA", bufs=2, space="PSUM"))
        psS = ctx.enter_context(tc.tile_pool(name="psS", bufs=2, space="PSUM"))
        psC = ctx.enter_context(tc.tile_pool(name="psC", bufs=2, space="PSUM"))

        x8_sb = pers.tile([128, KP, 2, T], FP8, tag="x8sb")
        x8l_sb = pers.tile([128, KP, 2, T], FP8, tag="x8lsb")
        wq8_sb = pers.tile([128, KP, 2, D], FP8, tag="wq8sb")
        wq8l_sb = pers.tile([128, KP, 2, D], FP8, tag="wq8lsb")
        wk8_sb = pers.tile([128, KP, 2, D], FP8, tag="wk8sb")
        wk8l_sb = pers.tile([128, KP, 2, D], FP8, tag="wk8lsb")
        wv8_sb = pers.tile([128, KP, 2, D], FP8, tag="wv8sb")
        wv8l_sb = pers.tile([128, KP, 2, D], FP8, tag="wv8lsb")
        dsc_sb = pers.tile([128, 2 * KT + 1], F32, tag="dscsb")
        kq_sb = pers.tile([128, 2, KT, T], BF16, tag="kqsb")
        v_sb = pers.tile([128, MT, VW], BF16, tag="vsb")
        out_sb = pers.tile([128, MT, D], BF16, tag="outsb")

        # ---- input DMAs, ordered by first use; first-needed tiles split out ----
        nc.sync.dma_start(out=dsc_sb, in_=dsc)

        def pack8(t):
            return t.rearrange("(kp half p) t -> p kp half t", kp=KP, half=2)

        qk_srcs = (
            (wq8_sb, wq8),
            (wk8_sb, wk8),
            (x8_sb, x8),
            (wq8l_sb, wq8l),
            (wk8l_sb, wk8l),
            (x8l_sb, x8l),
        )
        for kp in range(KP):
            for sb, dr in qk_srcs:
                nc.sync.dma_start(out=sb[:, kp], in_=pack8(dr)[:, kp])
        nc.sync.dma_start(out=wv8_sb, in_=pack8(wv8))
        nc.sync.dma_start(out=wv8l_sb, in_=pack8(wv8l))

        ae_tiles = {}

        def ae_load(h):
            w = ae_offs[h][JT - 1] + _spans(band[h])[JT - 1][1] - ae_offs[h][0]
            t = aexp_p.tile([128, w], BF16, tag="ae", name=f"ae{h}")
            nc.sync.dma_start(out=t, in_=ae[:, ae_offs[h][0] : ae_offs[h][0] + w])
            ae_tiles[h] = t

        for h in range(6):
            ae_load(h)

        # ones column per head for the fused rowsum
        v65 = v_sb.rearrange("p m (h c) -> p m h c", c=65)
        nc.vector.memset(v65[:, :, :, 64:65], 1.0)

        # ---- projections ----
        def proj_qk(pi, gi):
            """pi: 0=q (DVE descale copy), 1=k (ACT descale copy).
            3 fp8 rails: x8*w8 + x8*w8l + x8l*w8 (score noise ~0.1%)."""
            w_sb, wl_sb = (wq8_sb, wq8l_sb) if pi == 0 else (wk8_sb, wk8l_sb)
            rails = ((x8_sb, w_sb), (x8_sb, wl_sb), (x8l_sb, w_sb))
            dc = gi if pi == 0 else KT + gi
            for cc in range(2):
                pA = psA.tile([128, 2, 256], F32, tag="psA", name=f"pA{pi}{gi}{cc}")
                for ci in range(2):
                    c = 2 * cc + ci
                    n = 0
                    for kp in range(KP):
                        for xs, ws in rails:
                            nc.tensor.matmul(
                                pA[:, ci, :],
                                lhsT=ws[:, kp, :, gi * 128 : (gi + 1) * 128],
                                rhs=xs[:, kp, :, c * 256 : (c + 1) * 256],
                                start=(n == 0),
                                stop=(n == 3 * KP - 1),
                                perf_mode=mybir.MatmulPerfMode.DoubleRow,
                            )
                            n += 1
                dst = kq_sb[:, pi, gi, cc * 512 : (cc + 1) * 512]
                if pi == 0:
                    nc.vector.tensor_scalar_mul(
                        out=dst, in0=pA, scalar1=dsc_sb[:, dc : dc + 1]
                    )
                else:
                    nc.scalar.activation(
                        out=dst,
                        in_=pA,
                        func=mybir.ActivationFunctionType.Copy,
                        scale=dsc_sb[:, dc : dc + 1],
                    )

        def proj_v(mt, half):
            # dual-rail fp8 DoubleRow: v = x8*wv8 + x8*wv8l + x8l*wv8
            # (all rails at scale ax*aw; one uniform descale at the copy)
            c0, cw = (0, 512) if half == 0 else (512, 256)
            pV = psA.tile([128, 512], F32, tag="psA", name=f"pV{mt}{half}")
            rails = ((x8_sb, wv8_sb), (x8_sb, wv8l_sb), (x8l_sb, wv8_sb))
            for ci in range(cw // 256):
                c = c0 + ci * 256
                n = 0
                for xs, ws in rails:
                    for kp in range(KP):
                        nc.tensor.matmul(
                            pV[:, ci * 256 : ci * 256 + 256]
                            if half == 0
                            else pV[:, 0:256],
                            lhsT=xs[:, kp, :, mt * 128 : (mt + 1) * 128],
                            rhs=ws[:, kp, :, c : c + 256],
                            start=(n == 0),
                            stop=(n == 3 * KP - 1),
                            perf_mode=mybir.MatmulPerfMode.DoubleRow,
                        )
                        n += 1
            dv = dsc_sb[:, 2 * KT : 2 * KT + 1]
            if half == 0:
                nc.vector.tensor_scalar_mul(
                    out=v65[:, mt, 0:8, 0:64], in0=pV[:, :cw], scalar1=dv
                )
            else:
                nc.scalar.activation(
                    out=v65[:, mt, 8:12, 0:64],
                    in_=pV[:, 0:cw],
                    func=mybir.ActivationFunctionType.Copy,
                    scale=dv,
                )

        # ---- attention: scores phase ----
        eT_tiles = {}

        def att_sc(h, jts):
            gi, po = h // 2, (h % 2) * 64
            spans = _spans(band[h])
            # first pool rotation (h<6): eT buffers are fresh SBUF, so exp must
            # define the full tile span; afterwards stale-but-finite contents
            # let the ae=0 multiply zero-fill the fine-span gaps.
            fine = spans if h < 6 else _fine_spans(band[h], FINE_L[h])
            if h not in eT_tiles:
                eT_tiles[h] = exp_p.tile([128, JT, 2, S], BF16, tag="eT", name=f"eT{h}")
            eT = eT_tiles[h]
            for jt in jts:
                f0, fw = fine[jt]
                sc = psS.tile([128, 2, S], F32, tag="sc", name=f"sc{h}{jt}")
                for b in range(BPC):
                    t0 = b * S
                    nc.tensor.matmul(
                        sc[:, b, f0 : f0 + fw],
                        lhsT=kq_sb[
                            po : po + DH, 1, gi, t0 + jt * 128 : t0 + (jt + 1) * 128
                        ],
                        rhs=kq_sb[po : po + DH, 0, gi, t0 + f0 : t0 + f0 + fw],
                        start=True,
                        stop=True,
                    )
                nc.scalar.activation(
                    out=eT[:, jt, :, f0 : f0 + fw],
                    in_=sc[:, :, f0 : f0 + fw],
                    func=mybir.ActivationFunctionType.Exp,
                )

        def att_mul(h):
            spans = _spans(band[h])
            if 6 <= h + 2 < H:
                ae_load(h + 2)
            aeh = ae_tiles[h]
            ae0 = ae_offs[h][0]
            eT = eT_tiles[h]
            for jt in range(JT):
                i0, iw = spans[jt]
                off = ae_offs[h][jt] - ae0
                nc.vector.tensor_mul(
                    out=eT[:, jt, :, i0 : i0 + iw],
                    in0=eT[:, jt, :, i0 : i0 + iw],
                    in1=aeh[:, off : off + iw].unsqueeze(1).broadcast_to((128, 2, iw)),
                )

        # ---- attention: per-batch PV + normalize phase ----
        def att_pv(h, b):
            dt_h = band[h]
            eT = eT_tiles[h]
            pC = psC.tile([128, JT, 128], F32, tag="pC", name=f"pC{h}{b}")
            for it in range(JT):
                jts = [jt for jt in range(JT) if abs(jt - it) <= dt_h]
                for n, jt in enumerate(jts):
                    nc.tensor.matmul(
                        pC[:, it, 0:65],
                        lhsT=eT[:, jt, b, it * 128 : (it + 1) * 128],
                        rhs=v_sb[:, b * JT + jt, h * 65 : h * 65 + 65],
                        start=(n == 0),
                        stop=(n == len(jts) - 1),
                    )
            r = r_p.tile([128, JT], F32, tag="r", name=f"r{h}{b}")
            nc.vector.reciprocal(out=r, in_=pC[:, :, 64:65])
            nc.vector.tensor_mul(
                out=out_sb[:, b * JT : (b + 1) * JT, h * DH : (h + 1) * DH],
                in0=pC[:, :, 0:64],
                in1=r.unsqueeze(2).broadcast_to((128, JT, DH)),
            )

        def out_chunk(c0, cw):
            # column chunk across all token tiles: row (mt,p), cols c0:c0+cw
            nc.sync.dma_start(
                out=out.rearrange("(mt p) d -> p mt d", p=128)[:, :, c0 : c0 + cw],
                in_=out_sb[:, :, c0 : c0 + cw],
            )

        # ---- schedule: strict rounds. Per round r: project ft-tile r (its
        # psum-freeing copies land at the HEAD of the DVE/ACT queues, before
        # this round's muls), score heads 2r/2r+1 with the two heads' jt
        # tiles interleaved (hides exp latency behind the other head's
        # matmuls), multiply, then PV heads from two rounds back. ----
        def sc_pair(a):
            att_sc(a, (0, 1))
            att_sc(a + 1, (0, 1))
            att_sc(a, (2, 3))
            att_sc(a + 1, (2, 3))

        # rounds: projections run one ft-tile ahead of the scores that
        # consume them, so sc-matmuls never wait on same-round copies
        proj_qk(0, 0)
        proj_qk(1, 0)
        proj_qk(0, 1)
        proj_qk(1, 1)
        sc_pair(0)
        att_mul(0)
        att_mul(1)
        # r1
        proj_qk(0, 2)
        proj_qk(1, 2)
        sc_pair(2)
        for mt in range(4):
            proj_v(mt, 0)
        att_mul(2)
        att_mul(3)
        # r2
        proj_qk(0, 3)
        proj_qk(1, 3)
        sc_pair(4)
        for mt in range(4, MT):
            proj_v(mt, 0)
        att_mul(4)
        att_mul(5)
        att_pv(0, 0)
        att_pv(1, 0)
        att_pv(0, 1)
        att_pv(1, 1)
        # r3
        proj_qk(0, 4)
        proj_qk(1, 4)
        sc_pair(6)
        att_mul(6)
        att_mul(7)
        att_pv(2, 0)
        att_pv(2, 1)
        att_pv(3, 0)
        att_pv(3, 1)
        out_chunk(0, 256)
        # r4
        proj_qk(0, 5)
        proj_qk(1, 5)
        sc_pair(8)
        for mt in range(4):
            proj_v(mt, 1)
        att_mul(8)
        att_mul(9)
        att_pv(4, 0)
        att_pv(4, 1)
        att_pv(5, 0)
        att_pv(5, 1)
        # r5
        sc_pair(10)
        for mt in range(4, MT):
            proj_v(mt, 1)
        att_mul(10)
        att_mul(11)
        att_pv(6, 0)
        att_pv(6, 1)
        att_pv(7, 0)
        att_pv(7, 1)
        out_chunk(256, 256)
        att_pv(8, 0)
        att_pv(9, 0)
        # r6
        att_pv(8, 1)
        att_pv(9, 1)
        att_pv(10, 0)
        att_pv(10, 1)
        att_pv(11, 0)
        att_pv(11, 1)
        out_chunk(512, 256)
    _split_multi_waits(nc)
    return nc


def host_prep(inputs: dict, band=None):
    """Returns (shared input dict, per-core dict list)."""
    import ml_dtypes

    band = band or BAND
    ae_offs, ae_total = _ae_layout(band)

    hs = np.ascontiguousarray(np.asarray(inputs["hidden_states"], np.float32))
    Wq = np.asarray(inputs["Wq"], np.float32)
    Wk = np.asarray(inputs["Wk"], np.float32)
    Wv = np.asarray(inputs["Wv"], np.float32)
    qfc = np.asarray(inputs["query_fc"], np.float32)
    kfc = np.asarray(inputs["key_fc"], np.float32)
    mwt = np.asarray(inputs["mixture_weight"], np.float32)[0, :, 0, 0, :]  # [H,2]

    e = np.exp(mwt - mwt.max(-1, keepdims=True))
    mw = e / e.sum(-1, keepdims=True)
    qscale = np.repeat(mw[:, 0] / np.sqrt(DH), DH).astype(np.float32)  # per out col

    ax = FP8_MAX / max(np.abs(hs).max(), 1e-30)

    def quant_w(wT):  # wT [in_feat, out_feat] -> (fp8 array, descale per col)
        amax = np.abs(wT).max(axis=0)
        ac = FP8_MAX / np.maximum(amax, 1e-30)
        w8 = (wT * ac[None, :]).astype(ml_dtypes.float8_e4m3)
        return w8, (1.0 / (ax * ac)).astype(np.float32)

    def quant_w_rails(wT):
        w8, d = quant_w(wT)
        ac = FP8_MAX / np.maximum(np.abs(wT).max(axis=0), 1e-30)
        w8l = ((wT * ac[None, :]) - w8.astype(np.float32)).astype(
            ml_dtypes.float8_e4m3
        )
        return w8, w8l, d

    wq8, wq8l, dq = quant_w_rails(Wq.T * qscale[None, :])
    wk8, wk8l, dk = quant_w_rails(Wk.T)

    # dual-rail fp8 V weights at one global scale
    wvT = Wv.T.astype(np.float32)
    aw = FP8_MAX / max(np.abs(wvT).max(), 1e-30)
    wv8 = (wvT * aw).astype(ml_dtypes.float8_e4m3)
    wv8l = ((wvT - wv8.astype(np.float32) / aw) * aw).astype(ml_dtypes.float8_e4m3)
    dv = np.full((128, 1), 1.0 / (ax * aw), np.float32)

    dsc = np.concatenate(
        [dq.reshape(KT, 128).T, dk.reshape(KT, 128).T, dv], axis=1
    )  # [128, 2*KT+1]
    dsc = np.ascontiguousarray(dsc)

    # packed banded bias table: ae[h][p, off+e] = exp(mw1*synth^T + alibi)
    synthT = np.einsum("hik,hjk->hji", qfc, kfc).astype(np.float32)
    pos = np.arange(S)
    absd = np.abs(pos[None, :] - pos[:, None]).astype(np.float32)
    slopes = SLOPES.astype(np.float32)
    bias = mw[:, 1][:, None, None] * synthT - slopes[:, None, None] * absd[None]
    aexp = np.exp(bias)  # [H, S(j), S(i)]
    ae_pack = np.zeros((128, ae_total), np.float32)
    for h in range(H):
        sp = _spans(band[h])
        fine = _fine_spans(band[h], FINE_L[h])
        for jt in range(JT):
            i0, iw = sp[jt]
            blk = aexp[h, jt * 128 : (jt + 1) * 128, i0 : i0 + iw].copy()
            f0, fw = fine[jt]
            blk[:, : f0 - i0] = 0.0
            blk[:, f0 - i0 + fw :] = 0.0
            ae_pack[:, ae_offs[h][jt] : ae_offs[h][jt] + iw] = blk
    ae_pack = np.ascontiguousarray(ae_pack.astype(ml_dtypes.bfloat16))

    shared = dict(
        wq8=wq8, wq8l=wq8l, wk8=wk8, wk8l=wk8l, wv8=wv8, wv8l=wv8l, dsc=dsc, ae=ae_pack
    )
    n_cores = hs.shape[0] // BPC
    per_core = []
    for c in range(n_cores):
        xT = np.ascontiguousarray(hs[c * BPC : (c + 1) * BPC].reshape(T, D).T)
        x8 = (xT * ax).astype(ml_dtypes.float8_e4m3)
        x8l = ((xT - x8.astype(np.float32) / ax) * ax).astype(ml_dtypes.float8_e4m3)
        per_core.append(
            dict(x8=np.ascontiguousarray(x8), x8l=np.ascontiguousarray(x8l))
        )
    return shared, per_core


# ---------------------------------------------------------------------------
# Harness entry point: full (unsharded) inputs -> full output.
# Shards batch 16 -> 8 cores x 2, runs the SPMD Bass kernel, gathers.
# ---------------------------------------------------------------------------

N_CORES = 8
_NC_CACHE: dict = {}


def kernel(**inputs) -> np.ndarray:
    shared, per_core = host_prep(inputs)
    if "nc" not in _NC_CACHE:
        _NC_CACHE["nc"] = build_nc()
    nc = _NC_CACHE["nc"]
    in_maps = [dict(shared, **per_core[c]) for c in range(N_CORES)]
    from concourse.bass_utils import run_bass_kernel_spmd

    res = run_bass_kernel_spmd(nc, in_maps, core_ids=list(range(N_CORES)))
    outs = [
        res.results[c]["out"].astype(np.float32).reshape(BPC, S, D)
        for c in range(N_CORES)
    ]
    return np.concatenate(outs, axis=0)


# revision 53
# speedup vs baseline: 1.3866x; 1.0090x over previous
"""BertSelfAttention (synthesizer mixture + symmetric ALiBi) Bass kernel, TRN2.

Data-parallel over batch: 8 cores x 2 batches each. One SPMD program.

Per-core decomposition (b=0,1; heads h=0..11; token tiles of 128):
  host: mw = softmax(mixture_weight); fold mw0/sqrt(64) into Wq columns;
        ae[h,j,i] = exp(mw1*synth[h,i,j] - slope_h*|i-j|) (content-indep
        table, banded+packed, bf16);
        quantize Wq,Wk (per-column scale) and x (global scale) to fp8e4.
  PE:   qT,kT = W8.T @ x8 via fp8 DoubleRow matmuls (256-contraction,
        0.5 cyc/row); v = x @ WvT in bf16.
  DVE/ACT: psum->sbuf descale copies (per-partition 1/(ax*acol) scalars;
        ACT uses activation-Copy with scale).
  PE:   scT[j,i] = kT_h.T @ qT_h (bf16, 64-contraction), both batches into
        one 2-bank psum tile.
  ACT:  eT = exp(scT) for both batches in one instruction (no max
        subtraction: scores empirically bounded ~[-2.3, 2.3]).
  DVE:  eT *= ae (2x bf16 mode, ae broadcast across batches).
  PE:   [ctx | rowsum] = eT.T @ [v_h | 1] -- ones column interleaved in
        v layout (65 cols/head) so rowsum rides the same accumulation.
  ACT:  r = 1/rowsum (activation Reciprocal).
  DVE:  out = ctx * r, fused per (h,b) over all 4 row tiles.

ALiBi banding: (jt,it) tile pairs with negligible contribution skipped
(scores, exp, mul, pv, and the ae columns). attend() is split into a
scores phase and a per-batch PV phase so PE's in-order stream never
blocks on V projections; V-batch-0 PV only needs v rows 0..511.
"""

from contextlib import ExitStack

import numpy as np

import concourse.bass as bass
import concourse.mybir as mybir
import concourse.tile as tile

F32 = mybir.dt.float32
BF16 = mybir.dt.bfloat16
FP8 = mybir.dt.float8e4

H, S, D, DH = 12, 512, 768, 64
BPC = 2                # batches per core
T = BPC * S            # tokens per core
KT = D // 128          # bf16 contraction tiles over model dim
KP = D // 256          # fp8 DoubleRow contraction pair-tiles
MT = T // 128          # token tiles per core
JT = S // 128          # key tiles per sequence
VW = H * 65            # v columns incl. interleaved ones column per head
FP8_MAX = 192.0        # target max for float8_e4m3 (type max 240)

# max |jt-it| kept per head (dropped tile-pairs contribute <~1e-3 of rowsum)
BAND = [1, 1, 1, 1, 2, 3, 3, 3, 1, 1, 1, 1]
# fine alibi reach per head (exp/scores shrink to these spans; ae-table
# zeros + the tile-span multiply zero-fill the remainder)
FINE_L = [32, 32, 64, 128, 256, 99999, 99999, 99999, 32, 32, 64, 96]


def _get_slopes(n):
    import math

    def pow2(n):
        start = 2 ** (-(2 ** (-(math.log2(n) - 3))))
        return [start * start**i for i in range(n)]

    if math.log2(n).is_integer():
        return pow2(n)
    cp2 = 2 ** math.floor(math.log2(n))
    return pow2(cp2) + _get_slopes(2 * cp2)[0::2][: n - cp2]


SLOPES = np.asarray(_get_slopes(H), np.float64)


def _spans(dt_h):
    """Per jt: (i0, iw) kept column range (PV-tile granular)."""
    out = []
    for jt in range(JT):
        lo = max(0, jt - dt_h)
        hi = min(JT - 1, jt + dt_h)
        out.append((lo * 128, (hi - lo + 1) * 128))
    return out


def _fine_spans(dt_h, L):
    """Per jt: (i0, iw) 64-granular span [jt*128-L, (jt+1)*128+L) clipped to
    the tile span. Columns outside carry ae=0, so eT there is zero-filled by
    the multiply."""
    out = []
    for jt, (t0, tw) in enumerate(_spans(dt_h)):
        lo = max(t0, jt * 128 - L)
        hi = min(t0 + tw, (jt + 1) * 128 + L)
        out.append((lo, hi - lo))
    return out


def _ae_layout(band):
    """Per-head per-jt column offsets into the packed ae table."""
    offs, total = [], 0
    for h in range(H):
        sp = _spans(band[h])
        jt_off = []
        for jt in range(JT):
            jt_off.append(total)
            total += sp[jt][1]
        offs.append(jt_off)
    return offs, total


def _patch_tile_drain():
    """This walrus build rejects >1 sync-wait on one instruction; split the
    TileContext tail-drain's waits across single-wait drains."""
    from concourse.vector_clock import ScopedClock

    def _drain_and_barrier(self, tick_clock, wait_clock):
        nc = self.nc
        drain_inst = nc.sync.drain()
        wait_clock.add_sem_waits(
            drain_inst.ins, ScopedClock({None: tick_clock.global_clock})
        )
        waits = list(drain_inst.ins.sync_info.on_wait)
        if len(waits) > 1:
            drain_inst.ins.sync_info.on_wait = waits[:1]
            for w in waits[1:]:
                extra = nc.sync.drain()
                extra.ins.sync_info = mybir.SyncInfo(on_wait=[w], on_update=[])
        nc.all_engine_barrier()
        assert self.sems is not None
        popped = nc._tile_sem_poison_stack.pop()
        assert popped is self._sem_poison
        nc.clear_and_free_semaphores(list(self.sems.allocated().values()))
        nc.all_engine_barrier()

    tile.TileContext._drain_and_barrier = _drain_and_barrier


_patch_tile_drain()


def _split_multi_waits(nc):
    """This walrus build accepts at most one sync-wait per instruction; hoist
    extra waits onto single-wait NOPs emitted just before, on the same engine."""
    for fn in nc.m.functions:
        for bb in fn.blocks:
            out = []
            changed = False
            for ins in bb.instructions:
                si = ins.sync_info
                if si is not None and si.on_wait and len(si.on_wait) > 1:
                    waits = list(si.on_wait)
                    for i, w in enumerate(waits[:-1]):
                        nop = mybir.InstNoOp(
                            name=f"{ins.name}_w{i}",
                            engine=ins.engine,
                            sync_info=mybir.SyncInfo(on_wait=[w], on_update=[]),
                            bass_nofuse=True,
                        )
                        nc.register_instruction(nop, overwrite=True)
                        out.append(nop)
                    si.on_wait = waits[-1:]
                    changed = True
                out.append(ins)
            if changed:
                bb.instructions = out


def build_nc(band=None) -> bass.Bass:
    band = band or BAND
    ae_offs, ae_total = _ae_layout(band)
    nc = bass.Bass("TRN2")
    x8 = nc.dram_tensor("x8", [D, T], FP8, kind="ExternalInput").ap()
    x8l = nc.dram_tensor("x8l", [D, T], FP8, kind="ExternalInput").ap()
    wq8 = nc.dram_tensor("wq8", [D, D], FP8, kind="ExternalInput").ap()
    wq8l = nc.dram_tensor("wq8l", [D, D], FP8, kind="ExternalInput").ap()
    wk8 = nc.dram_tensor("wk8", [D, D], FP8, kind="ExternalInput").ap()
    wk8l = nc.dram_tensor("wk8l", [D, D], FP8, kind="ExternalInput").ap()
    wv8 = nc.dram_tensor("wv8", [D, D], FP8, kind="ExternalInput").ap()
    wv8l = nc.dram_tensor("wv8l", [D, D], FP8, kind="ExternalInput").ap()
    dsc = nc.dram_tensor("dsc", [128, 2 * KT + 1], F32, kind="ExternalInput").ap()
    ae = nc.dram_tensor("ae", [128, ae_total], BF16, kind="ExternalInput").ap()
    out = nc.dram_tensor("out", [T, D], BF16, kind="ExternalOutput").ap()

    with tile.TileContext(nc) as tc, ExitStack() as ctx:
        pers = ctx.enter_context(tc.tile_pool(name="pers", bufs=1))
        aexp_p = ctx.enter_context(tc.tile_pool(name="aexp_p", bufs=6))
        exp_p = ctx.enter_context(tc.tile_pool(name="exp_p", bufs=8))
        r_p = ctx.enter_context(tc.tile_pool(name="r_p", bufs=2))
        psA = ctx.enter_context(tc.tile_pool(name="psA", bufs=2, space="PSUM"))
        psS = ctx.enter_context(tc.tile_pool(name="psS", bufs=2, space="PSUM"))
        psC = ctx.enter_context(tc.tile_pool(name="psC", bufs=2, space="PSUM"))

        x8_sb = pers.tile([128, KP, 2, T], FP8, tag="x8sb")
        x8l_sb = pers.tile([128, KP, 2, T], FP8, tag="x8lsb")
        wq8_sb = pers.tile([128, KP, 2, D], FP8, tag="wq8sb")
        wq8l_sb = pers.tile([128, KP, 2, D], FP8, tag="wq8lsb")
        wk8_sb = pers.tile([128, KP, 2, D], FP8, tag="wk8sb")
        wk8l_sb = pers.tile([128, KP, 2, D], FP8, tag="wk8lsb")
        wv8_sb = pers.tile([128, KP, 2, D], FP8, tag="wv8sb")
        wv8l_sb = pers.tile([128, KP, 2, D], FP8, tag="wv8lsb")
        dsc_sb = pers.tile([128, 2 * KT + 1], F32, tag="dscsb")
        kq_sb = pers.tile([128, 2, KT, T], BF16, tag="kqsb")
        v_sb = pers.tile([128, MT, VW], BF16, tag="vsb")
        out_sb = pers.tile([128, MT, D], BF16, tag="outsb")

        # ---- input DMAs, ordered by first use; first-needed tiles split out ----
        nc.sync.dma_start(out=dsc_sb, in_=dsc)

        def pack8(t):
            return t.rearrange("(kp half p) t -> p kp half t", kp=KP, half=2)

        qk_srcs = (
            (wq8_sb, wq8),
            (wk8_sb, wk8),
            (x8_sb, x8),
            (wq8l_sb, wq8l),
            (wk8l_sb, wk8l),
            (x8l_sb, x8l),
        )
        for kp in range(KP):
            for sb, dr in qk_srcs:
                nc.sync.dma_start(out=sb[:, kp], in_=pack8(dr)[:, kp])
        nc.sync.dma_start(out=wv8_sb, in_=pack8(wv8))
        nc.sync.dma_start(out=wv8l_sb, in_=pack8(wv8l))

        ae_tiles = {}

        def ae_load(h):
            w = ae_offs[h][JT - 1] + _spans(band[h])[JT - 1][1] - ae_offs[h][0]
            t = aexp_p.tile([128, w], BF16, tag="ae", name=f"ae{h}")
            nc.sync.dma_start(out=t, in_=ae[:, ae_offs[h][0] : ae_offs[h][0] + w])
            ae_tiles[h] = t

        for h in range(6):
            ae_load(h)

        # ones column per head for the fused rowsum
        v65 = v_sb.rearrange("p m (h c) -> p m h c", c=65)
        nc.vector.memset(v65[:, :, :, 64:65], 1.0)

        # ---- projections ----
        def proj_qk(pi, gi):
            """pi: 0=q (DVE descale copy), 1=k (ACT descale copy).
            3 fp8 rails: x8*w8 + x8*w8l + x8l*w8 (score noise ~0.1%)."""
            w_sb, wl_sb = (wq8_sb, wq8l_sb) if pi == 0 else (wk8_sb, wk8l_sb)
            rails = ((x8_sb, w_sb), (x8_sb, wl_sb), (x8l_sb, w_sb))
            dc = gi if pi == 0 else KT + gi
            for cc in range(2):
                pA = psA.tile([128, 2, 256], F32, tag="psA", name=f"pA{pi}{gi}{cc}")
                for ci in range(2):
                    c = 2 * cc + ci
                    n = 0
                    for kp in range(KP):
                        for xs, ws in rails:
                            nc.tensor.matmul(
                                pA[:, ci, :],
                                lhsT=ws[:, kp, :, gi * 128 : (gi + 1) * 128],
                                rhs=xs[:, kp, :, c * 256 : (c + 1) * 256],
                                start=(n == 0),
                                stop=(n == 3 * KP - 1),
                                perf_mode=mybir.MatmulPerfMode.DoubleRow,
                            )
                            n += 1
                dst = kq_sb[:, pi, gi, cc * 512 : (cc + 1) * 512]
                if pi == 0:
                    nc.vector.tensor_scalar_mul(
                        out=dst, in0=pA, scalar1=dsc_sb[:, dc : dc + 1]
                    )
                else:
                    nc.scalar.activation(
                        out=dst,
                        in_=pA,
                        func=mybir.ActivationFunctionType.Copy,
                        scale=dsc_sb[:, dc : dc + 1],
                    )

        def proj_v(mt, half):
            # dual-rail fp8 DoubleRow: v = x8*wv8 + x8*wv8l + x8l*wv8
            # (all rails at scale ax*aw; one uniform descale at the copy)
            c0, cw = (0, 512) if half == 0 else (512, 256)
            pV = psA.tile([128, 512], F32, tag="psA", name=f"pV{mt}{half}")
            rails = ((x8_sb, wv8_sb), (x8_sb, wv8l_sb), (x8l_sb, wv8_sb))
            for ci in range(cw // 256):
                c = c0 + ci * 256
                n = 0
                for xs, ws in rails:
                    for kp in range(KP):
                        nc.tensor.matmul(
                            pV[:, ci * 256 : ci * 256 + 256]
                            if half == 0
                            else pV[:, 0:256],
                            lhsT=xs[:, kp, :, mt * 128 : (mt + 1) * 128],
                            rhs=ws[:, kp, :, c : c + 256],
                            start=(n == 0),
                            stop=(n == 3 * KP - 1),
                            perf_mode=mybir.MatmulPerfMode.DoubleRow,
                        )
                        n += 1
            dv = dsc_sb[:, 2 * KT : 2 * KT + 1]
            if half == 0:
                nc.vector.tensor_scalar_mul(
                    out=v65[:, mt, 0:8, 0:64], in0=pV[:, :cw], scalar1=dv
                )
            else:
                nc.scalar.activation(
                    out=v65[:, mt, 8:12, 0:64],
                    in_=pV[:, 0:cw],
                    func=mybir.ActivationFunctionType.Copy,
                    scale=dv,
                )

        # ---- attention: scores phase ----
        eT_tiles = {}

        def att_sc(h, jts):
            gi, po = h // 2, (h % 2) * 64
            spans = _spans(band[h])
            # first pool rotation (h<6): eT buffers are fresh SBUF, so exp must
            # define the full tile span; afterwards stale-but-finite contents
            # let the ae=0 multiply zero-fill the fine-span gaps.
            fine = spans if h < 6 else _fine_spans(band[h], FINE_L[h])
            if h not in eT_tiles:
                eT_tiles[h] = exp_p.tile([128, JT, 2, S], BF16, tag="eT", name=f"eT{h}")
            eT = eT_tiles[h]
            for jt in jts:
                f0, fw = fine[jt]
                sc = psS.tile([128, 2, S], F32, tag="sc", name=f"sc{h}{jt}")
                for b in range(BPC):
                    t0 = b * S
                    nc.tensor.matmul(
                        sc[:, b, f0 : f0 + fw],
                        lhsT=kq_sb[
                            po : po + DH, 1, gi, t0 + jt * 128 : t0 + (jt + 1) * 128
                        ],
                        rhs=kq_sb[po : po + DH, 0, gi, t0 + f0 : t0 + f0 + fw],
                        start=True,
                        stop=True,
                    )
                nc.scalar.activation(
                    out=eT[:, jt, :, f0 : f0 + fw],
                    in_=sc[:, :, f0 : f0 + fw],
                    func=mybir.ActivationFunctionType.Exp,
                )

        def att_mul(h):
            spans = _spans(band[h])
            if 6 <= h + 2 < H:
                ae_load(h + 2)
            aeh = ae_tiles[h]
            ae0 = ae_offs[h][0]
            eT = eT_tiles[h]
            for jt in range(JT):
                i0, iw = spans[jt]
                off = ae_offs[h][jt] - ae0
                nc.vector.tensor_mul(
                    out=eT[:, jt, :, i0 : i0 + iw],
                    in0=eT[:, jt, :, i0 : i0 + iw],
                    in1=aeh[:, off : off + iw].unsqueeze(1).broadcast_to((128, 2, iw)),
                )

        # ---- attention: per-batch PV + normalize phase ----
        def att_pv(h, b):
            dt_h = band[h]
            eT = eT_tiles[h]
            pC = psC.tile([128, JT, 128], F32, tag="pC", name=f"pC{h}{b}")
            for it in range(JT):
                jts = [jt for jt in range(JT) if abs(jt - it) <= dt_h]
                for n, jt in enumerate(jts):
                    nc.tensor.matmul(
                        pC[:, it, 0:65],
                        lhsT=eT[:, jt, b, it * 128 : (it + 1) * 128],
                        rhs=v_sb[:, b * JT + jt, h * 65 : h * 65 + 65],
                        start=(n == 0),
                        stop=(n == len(jts) - 1),
                    )
            r = r_p.tile([128, JT], F32, tag="r", name=f"r{h}{b}")
            nc.vector.reciprocal(out=r, in_=pC[:, :, 64:65])
            nc.vector.tensor_mul(
                out=out_sb[:, b * JT : (b + 1) * JT, h * DH : (h + 1) * DH],
                in0=pC[:, :, 0:64],
                in1=r.unsqueeze(2).broadcast_to((128, JT, DH)),
            )

        def out_chunk(c0, cw):
            # column chunk across all token tiles: row (mt,p), cols c0:c0+cw
            nc.sync.dma_start(
                out=out.rearrange("(mt p) d -> p mt d", p=128)[:, :, c0 : c0 + cw],
                in_=out_sb[:, :, c0 : c0 + cw],
            )

        # ---- schedule: strict rounds. Per round r: project ft-tile r (its
        # psum-freeing copies land at the HEAD of the DVE/ACT queues, before
        # this round's muls), score heads 2r/2r+1 with the two heads' jt
        # tiles interleaved (hides exp latency behind the other head's
        # matmuls), multiply, then PV heads from two rounds back. ----
        def sc_pair(a):
            att_sc(a, (0, 1))
            att_sc(a + 1, (0, 1))
            att_sc(a, (2, 3))
            att_sc(a + 1, (2, 3))

        # rounds: projections run one ft-tile ahead of the scores that
        # consume them, so sc-matmuls never wait on same-round copies
        proj_qk(0, 0)
        proj_qk(1, 0)
        proj_qk(0, 1)
        proj_qk(1, 1)
        sc_pair(0)
        att_mul(0)
        att_mul(1)
        # r1
        proj_qk(0, 2)
        proj_qk(1, 2)
        sc_pair(2)
        for mt in range(4):
            proj_v(mt, 0)
        att_mul(2)
        att_mul(3)
        # r2
        proj_qk(0, 3)
        proj_qk(1, 3)
        sc_pair(4)
        for mt in range(4, MT):
            proj_v(mt, 0)
        att_mul(4)
        att_mul(5)
        att_pv(0, 0)
        att_pv(1, 0)
        att_pv(0, 1)
        att_pv(1, 1)
        # r3
        proj_qk(0, 4)
        proj_qk(1, 4)
        sc_pair(6)
        att_mul(6)
        att_mul(7)
        att_pv(2, 0)
        att_pv(2, 1)
        att_pv(3, 0)
        att_pv(3, 1)
        out_chunk(0, 256)
        # r4
        proj_qk(0, 5)
        proj_qk(1, 5)
        sc_pair(8)
        for mt in range(4):
            proj_v(mt, 1)
        att_mul(8)
        att_mul(9)
        att_pv(4, 0)
        att_pv(4, 1)
        att_pv(5, 0)
        att_pv(5, 1)
        # r5
        sc_pair(10)
        for mt in range(4, MT):
            proj_v(mt, 1)
        att_mul(10)
        att_mul(11)
        att_pv(6, 0)
        att_pv(6, 1)
        att_pv(7, 0)
        att_pv(7, 1)
        out_chunk(256, 256)
        att_pv(8, 0)
        att_pv(9, 0)
        att_pv(10, 0)
        att_pv(11, 0)
        nc.sync.dma_start(
            out=out.rearrange("(mt p) d -> p mt d", p=128)[:, 0:4, 512:768],
            in_=out_sb[:, 0:4, 512:768],
        )
        # r6
        att_pv(8, 1)
        att_pv(9, 1)
        att_pv(10, 1)
        att_pv(11, 1)
        nc.sync.dma_start(
            out=out.rearrange("(mt p) d -> p mt d", p=128)[:, 4:8, 512:768],
            in_=out_sb[:, 4:8, 512:768],
        )
    _split_multi_waits(nc)
    return nc


def host_prep(inputs: dict, band=None):
    """Returns (shared input dict, per-core dict list)."""
    import ml_dtypes

    band = band or BAND
    ae_offs, ae_total = _ae_layout(band)

    hs = np.ascontiguousarray(np.asarray(inputs["hidden_states"], np.float32))
    Wq = np.asarray(inputs["Wq"], np.float32)
    Wk = np.asarray(inputs["Wk"], np.float32)
    Wv = np.asarray(inputs["Wv"], np.float32)
    qfc = np.asarray(inputs["query_fc"], np.float32)
    kfc = np.asarray(inputs["key_fc"], np.float32)
    mwt = np.asarray(inputs["mixture_weight"], np.float32)[0, :, 0, 0, :]  # [H,2]

    e = np.exp(mwt - mwt.max(-1, keepdims=True))
    mw = e / e.sum(-1, keepdims=True)
    qscale = np.repeat(mw[:, 0] / np.sqrt(DH), DH).astype(np.float32)  # per out col

    ax = FP8_MAX / max(np.abs(hs).max(), 1e-30)

    def quant_w(wT):  # wT [in_feat, out_feat] -> (fp8 array, descale per col)
        amax = np.abs(wT).max(axis=0)
        ac = FP8_MAX / np.maximum(amax, 1e-30)
        w8 = (wT * ac[None, :]).astype(ml_dtypes.float8_e4m3)
        return w8, (1.0 / (ax * ac)).astype(np.float32)

    def quant_w_rails(wT):
        w8, d = quant_w(wT)
        ac = FP8_MAX / np.maximum(np.abs(wT).max(axis=0), 1e-30)
        w8l = ((wT * ac[None, :]) - w8.astype(np.float32)).astype(
            ml_dtypes.float8_e4m3
        )
        return w8, w8l, d

    wq8, wq8l, dq = quant_w_rails(Wq.T * qscale[None, :])
    wk8, wk8l, dk = quant_w_rails(Wk.T)

    # dual-rail fp8 V weights at one global scale
    wvT = Wv.T.astype(np.float32)
    aw = FP8_MAX / max(np.abs(wvT).max(), 1e-30)
    wv8 = (wvT * aw).astype(ml_dtypes.float8_e4m3)
    wv8l = ((wvT - wv8.astype(np.float32) / aw) * aw).astype(ml_dtypes.float8_e4m3)
    dv = np.full((128, 1), 1.0 / (ax * aw), np.float32)

    dsc = np.concatenate(
        [dq.reshape(KT, 128).T, dk.reshape(KT, 128).T, dv], axis=1
    )  # [128, 2*KT+1]
    dsc = np.ascontiguousarray(dsc)

    # packed banded bias table: ae[h][p, off+e] = exp(mw1*synth^T + alibi)
    synthT = np.einsum("hik,hjk->hji", qfc, kfc).astype(np.float32)
    pos = np.arange(S)
    absd = np.abs(pos[None, :] - pos[:, None]).astype(np.float32)
    slopes = SLOPES.astype(np.float32)
    bias = mw[:, 1][:, None, None] * synthT - slopes[:, None, None] * absd[None]
    aexp = np.exp(bias)  # [H, S(j), S(i)]
    ae_pack = np.zeros((128, ae_total), np.float32)
    for h in range(H):
        sp = _spans(band[h])
        fine = _fine_spans(band[h], FINE_L[h])
        for jt in range(JT):
            i0, iw = sp[jt]
            blk = aexp[h, jt * 128 : (jt + 1) * 128, i0 : i0 + iw].copy()
            f0, fw = fine[jt]
            blk[:, : f0 - i0] = 0.0
            blk[:, f0 - i0 + fw :] = 0.0
            ae_pack[:, ae_offs[h][jt] : ae_offs[h][jt] + iw] = blk
    ae_pack = np.ascontiguousarray(ae_pack.astype(ml_dtypes.bfloat16))

    shared = dict(
        wq8=wq8, wq8l=wq8l, wk8=wk8, wk8l=wk8l, wv8=wv8, wv8l=wv8l, dsc=dsc, ae=ae_pack
    )
    n_cores = hs.shape[0] // BPC
    per_core = []
    for c in range(n_cores):
        xT = np.ascontiguousarray(hs[c * BPC : (c + 1) * BPC].reshape(T, D).T)
        x8 = (xT * ax).astype(ml_dtypes.float8_e4m3)
        x8l = ((xT - x8.astype(np.float32) / ax) * ax).astype(ml_dtypes.float8_e4m3)
        per_core.append(
            dict(x8=np.ascontiguousarray(x8), x8l=np.ascontiguousarray(x8l))
        )
    return shared, per_core


# ---------------------------------------------------------------------------
# Harness entry point: full (unsharded) inputs -> full output.
# Shards batch 16 -> 8 cores x 2, runs the SPMD Bass kernel, gathers.
# ---------------------------------------------------------------------------

N_CORES = 8
_NC_CACHE: dict = {}


def kernel(**inputs) -> np.ndarray:
    shared, per_core = host_prep(inputs)
    if "nc" not in _NC_CACHE:
        _NC_CACHE["nc"] = build_nc()
    nc = _NC_CACHE["nc"]
    in_maps = [dict(shared, **per_core[c]) for c in range(N_CORES)]
    from concourse.bass_utils import run_bass_kernel_spmd

    res = run_bass_kernel_spmd(nc, in_maps, core_ids=list(range(N_CORES)))
    outs = [
        res.results[c]["out"].astype(np.float32).reshape(BPC, S, D)
        for c in range(N_CORES)
    ]
    return np.concatenate(outs, axis=0)


# revision 63
# speedup vs baseline: 1.4380x; 1.0370x over previous
"""BertSelfAttention (synthesizer mixture + symmetric ALiBi) Bass kernel, TRN2.

Data-parallel over batch: 8 cores x 2 batches each. One SPMD program.

Per-core decomposition (b=0,1; heads h=0..11; token tiles of 128):
  host: mw = softmax(mixture_weight); fold mw0/sqrt(64) into Wq columns;
        ae[h,j,i] = exp(mw1*synth[h,i,j] - slope_h*|i-j|) (content-indep
        table, banded+packed, bf16);
        quantize Wq,Wk (per-column scale) and x (global scale) to fp8e4.
  PE:   qT,kT = W8.T @ x8 via fp8 DoubleRow matmuls (256-contraction,
        0.5 cyc/row); v = x @ WvT in bf16.
  DVE/ACT: psum->sbuf descale copies (per-partition 1/(ax*acol) scalars;
        ACT uses activation-Copy with scale).
  PE:   scT[j,i] = kT_h.T @ qT_h (bf16, 64-contraction), both batches into
        one 2-bank psum tile.
  ACT:  eT = exp(scT) for both batches in one instruction (no max
        subtraction: scores empirically bounded ~[-2.3, 2.3]).
  DVE:  eT *= ae (2x bf16 mode, ae broadcast across batches).
  PE:   [ctx | rowsum] = eT.T @ [v_h | 1] -- ones column interleaved in
        v layout (65 cols/head) so rowsum rides the same accumulation.
  ACT:  r = 1/rowsum (activation Reciprocal).
  DVE:  out = ctx * r, fused per (h,b) over all 4 row tiles.

ALiBi banding: (jt,it) tile pairs with negligible contribution skipped
(scores, exp, mul, pv, and the ae columns). attend() is split into a
scores phase and a per-batch PV phase so PE's in-order stream never
blocks on V projections; V-batch-0 PV only needs v rows 0..511.
"""

from contextlib import ExitStack

import numpy as np

import concourse.bass as bass
import concourse.mybir as mybir
import concourse.tile as tile

F32 = mybir.dt.float32
BF16 = mybir.dt.bfloat16
FP8 = mybir.dt.float8e4

H, S, D, DH = 12, 512, 768, 64
BPC = 2                # batches per core
T = BPC * S            # tokens per core
KT = D // 128          # bf16 contraction tiles over model dim
KP = D // 256          # fp8 DoubleRow contraction pair-tiles
MT = T // 128          # token tiles per core
JT = S // 128          # key tiles per sequence
VW = H * 65            # v columns incl. interleaved ones column per head
FP8_MAX = 192.0        # target max for float8_e4m3 (type max 240)

# max |jt-it| kept per head (dropped tile-pairs contribute <~1e-3 of rowsum)
BAND = [1, 1, 1, 1, 2, 3, 3, 3, 1, 1, 1, 1]
# fine alibi reach per head (exp/scores shrink to these spans; ae-table
# zeros + the tile-span multiply zero-fill the remainder)
FINE_L = [32, 32, 64, 128, 256, 99999, 99999, 99999, 32, 32, 64, 96]


def _get_slopes(n):
    import math

    def pow2(n):
        start = 2 ** (-(2 ** (-(math.log2(n) - 3))))
        return [start * start**i for i in range(n)]

    if math.log2(n).is_integer():
        return pow2(n)
    cp2 = 2 ** math.floor(math.log2(n))
    return pow2(cp2) + _get_slopes(2 * cp2)[0::2][: n - cp2]


SLOPES = np.asarray(_get_slopes(H), np.float64)


def _spans(dt_h):
    """Per jt: (i0, iw) kept column range (PV-tile granular)."""
    out = []
    for jt in range(JT):
        lo = max(0, jt - dt_h)
        hi = min(JT - 1, jt + dt_h)
        out.append((lo * 128, (hi - lo + 1) * 128))
    return out


def _fine_spans(dt_h, L):
    """Per jt: (i0, iw) 64-granular span [jt*128-L, (jt+1)*128+L) clipped to
    the tile span. Columns outside carry ae=0, so eT there is zero-filled by
    the multiply."""
    out = []
    for jt, (t0, tw) in enumerate(_spans(dt_h)):
        lo = max(t0, jt * 128 - L)
        hi = min(t0 + tw, (jt + 1) * 128 + L)
        out.append((lo, hi - lo))
    return out


def _ae_layout(band):
    """Per-head per-jt column offsets into the packed ae table."""
    offs, total = [], 0
    for h in range(H):
        sp = _spans(band[h])
        jt_off = []
        for jt in range(JT):
            jt_off.append(total)
            total += sp[jt][1]
        offs.append(jt_off)
    return offs, total


def _patch_tile_drain():
    """This walrus build rejects >1 sync-wait on one instruction; split the
    TileContext tail-drain's waits across single-wait drains."""
    from concourse.vector_clock import ScopedClock

    def _drain_and_barrier(self, tick_clock, wait_clock):
        nc = self.nc
        drain_inst = nc.sync.drain()
        wait_clock.add_sem_waits(
            drain_inst.ins, ScopedClock({None: tick_clock.global_clock})
        )
        waits = list(drain_inst.ins.sync_info.on_wait)
        if len(waits) > 1:
            drain_inst.ins.sync_info.on_wait = waits[:1]
            for w in waits[1:]:
                extra = nc.sync.drain()
                extra.ins.sync_info = mybir.SyncInfo(on_wait=[w], on_update=[])
        nc.all_engine_barrier()
        assert self.sems is not None
        popped = nc._tile_sem_poison_stack.pop()
        assert popped is self._sem_poison
        nc.clear_and_free_semaphores(list(self.sems.allocated().values()))
        nc.all_engine_barrier()

    tile.TileContext._drain_and_barrier = _drain_and_barrier


_patch_tile_drain()


def _split_multi_waits(nc):
    """This walrus build accepts at most one sync-wait per instruction; hoist
    extra waits onto single-wait NOPs emitted just before, on the same engine."""
    for fn in nc.m.functions:
        for bb in fn.blocks:
            out = []
            changed = False
            for ins in bb.instructions:
                si = ins.sync_info
                if si is not None and si.on_wait and len(si.on_wait) > 1:
                    waits = list(si.on_wait)
                    for i, w in enumerate(waits[:-1]):
                        nop = mybir.InstNoOp(
                            name=f"{ins.name}_w{i}",
                            engine=ins.engine,
                            sync_info=mybir.SyncInfo(on_wait=[w], on_update=[]),
                            bass_nofuse=True,
                        )
                        nc.register_instruction(nop, overwrite=True)
                        out.append(nop)
                    si.on_wait = waits[-1:]
                    changed = True
                out.append(ins)
            if changed:
                bb.instructions = out


def build_nc(band=None) -> bass.Bass:
    band = band or BAND
    ae_offs, ae_total = _ae_layout(band)
    nc = bass.Bass("TRN2")
    x8 = nc.dram_tensor("x8", [D, T], FP8, kind="ExternalInput").ap()
    x8l = nc.dram_tensor("x8l", [D, T], FP8, kind="ExternalInput").ap()
    wq8 = nc.dram_tensor("wq8", [D, D], FP8, kind="ExternalInput").ap()
    wq8l = nc.dram_tensor("wq8l", [D, D], FP8, kind="ExternalInput").ap()
    wk8 = nc.dram_tensor("wk8", [D, D], FP8, kind="ExternalInput").ap()
    wk8l = nc.dram_tensor("wk8l", [D, D], FP8, kind="ExternalInput").ap()
    wv8 = nc.dram_tensor("wv8", [D, D], FP8, kind="ExternalInput").ap()
    wv8l = nc.dram_tensor("wv8l", [D, D], FP8, kind="ExternalInput").ap()
    dsc = nc.dram_tensor("dsc", [128, 2 * KT + 1], F32, kind="ExternalInput").ap()
    ae = nc.dram_tensor("ae", [128, ae_total], BF16, kind="ExternalInput").ap()
    out = nc.dram_tensor("out", [T, D], BF16, kind="ExternalOutput").ap()

    with tile.TileContext(nc) as tc, ExitStack() as ctx:
        pers = ctx.enter_context(tc.tile_pool(name="pers", bufs=1))
        aexp_p = ctx.enter_context(tc.tile_pool(name="aexp_p", bufs=6))
        exp_p = ctx.enter_context(tc.tile_pool(name="exp_p", bufs=8))
        r_p = ctx.enter_context(tc.tile_pool(name="r_p", bufs=2))
        psA = ctx.enter_context(tc.tile_pool(name="psA", bufs=2, space="PSUM"))
        psS = ctx.enter_context(tc.tile_pool(name="psS", bufs=2, space="PSUM"))
        psC = ctx.enter_context(tc.tile_pool(name="psC", bufs=2, space="PSUM"))

        x8_sb = pers.tile([128, KP, 2, T], FP8, tag="x8sb")
        x8l_sb = pers.tile([128, KP, 2, T], FP8, tag="x8lsb")
        wq8_sb = pers.tile([128, KP, 2, D], FP8, tag="wq8sb")
        wq8l_sb = pers.tile([128, KP, 2, D], FP8, tag="wq8lsb")
        wk8_sb = pers.tile([128, KP, 2, D], FP8, tag="wk8sb")
        wk8l_sb = pers.tile([128, KP, 2, D], FP8, tag="wk8lsb")
        wv8_sb = pers.tile([128, KP, 2, D], FP8, tag="wv8sb")
        wv8l_sb = pers.tile([128, KP, 2, D], FP8, tag="wv8lsb")
        dsc_sb = pers.tile([128, 2 * KT + 1], F32, tag="dscsb")
        kq_sb = pers.tile([128, 2, KT, T], BF16, tag="kqsb")
        v_sb = pers.tile([128, MT, VW], BF16, tag="vsb")
        out_sb = pers.tile([128, MT, D], BF16, tag="outsb")

        # ---- input DMAs, ordered by first use; first-needed tiles split out ----
        nc.sync.dma_start(out=dsc_sb, in_=dsc)

        def pack8(t):
            return t.rearrange("(kp half p) t -> p kp half t", kp=KP, half=2)

        qk_srcs = (
            (wq8_sb, wq8),
            (wk8_sb, wk8),
            (x8_sb, x8),
            (wq8l_sb, wq8l),
            (wk8l_sb, wk8l),
            (x8l_sb, x8l),
        )
        for kp in range(KP):
            for sb, dr in qk_srcs:
                nc.sync.dma_start(out=sb[:, kp], in_=pack8(dr)[:, kp])
        nc.sync.dma_start(out=wv8_sb, in_=pack8(wv8))
        nc.sync.dma_start(out=wv8l_sb, in_=pack8(wv8l))

        ae_tiles = {}

        def ae_load(h):
            w = ae_offs[h][JT - 1] + _spans(band[h])[JT - 1][1] - ae_offs[h][0]
            t = aexp_p.tile([128, w], BF16, tag="ae", name=f"ae{h}")
            nc.sync.dma_start(out=t, in_=ae[:, ae_offs[h][0] : ae_offs[h][0] + w])
            ae_tiles[h] = t

        for h in range(6):
            ae_load(h)

        # ones column per head for the fused rowsum
        v65 = v_sb.rearrange("p m (h c) -> p m h c", c=65)
        nc.vector.memset(v65[:, :, :, 64:65], 1.0)

        # ---- projections ----
        def proj_qk(pi, gi, pool=None):
            """pi: 0=q (DVE descale copy), 1=k (ACT descale copy).
            3 fp8 rails: x8*w8 + x8*w8l + x8l*w8 (score noise ~0.1%)."""
            w_sb, wl_sb = (wq8_sb, wq8l_sb) if pi == 0 else (wk8_sb, wk8l_sb)
            rails = ((x8_sb, w_sb), (x8_sb, wl_sb), (x8l_sb, w_sb))
            dc = gi if pi == 0 else KT + gi
            for cc in range(2):
                pool = pool or psA
                tag = {id(psA): "psA", id(psS): "sc", id(psC): "pC"}[id(pool)]
                pA = pool.tile([128, 2, 256], F32, tag=tag, name=f"pA{pi}{gi}{cc}")
                for ci in range(2):
                    c = 2 * cc + ci
                    n = 0
                    for kp in range(KP):
                        for xs, ws in rails:
                            nc.tensor.matmul(
                                pA[:, ci, :],
                                lhsT=ws[:, kp, :, gi * 128 : (gi + 1) * 128],
                                rhs=xs[:, kp, :, c * 256 : (c + 1) * 256],
                                start=(n == 0),
                                stop=(n == 3 * KP - 1),
                                perf_mode=mybir.MatmulPerfMode.DoubleRow,
                            )
                            n += 1
                dst = kq_sb[:, pi, gi, cc * 512 : (cc + 1) * 512]
                if pi == 0:
                    nc.vector.tensor_scalar_mul(
                        out=dst, in0=pA, scalar1=dsc_sb[:, dc : dc + 1]
                    )
                else:
                    nc.scalar.activation(
                        out=dst,
                        in_=pA,
                        func=mybir.ActivationFunctionType.Copy,
                        scale=dsc_sb[:, dc : dc + 1],
                    )

        def proj_v(mt, half):
            # dual-rail fp8 DoubleRow: v = x8*wv8 + x8*wv8l + x8l*wv8
            # (all rails at scale ax*aw; one uniform descale at the copy)
            c0, cw = (0, 512) if half == 0 else (512, 256)
            pV = psA.tile([128, 512], F32, tag="psA", name=f"pV{mt}{half}")
            rails = ((x8_sb, wv8_sb), (x8_sb, wv8l_sb), (x8l_sb, wv8_sb))
            for ci in range(cw // 256):
                c = c0 + ci * 256
                n = 0
                for xs, ws in rails:
                    for kp in range(KP):
                        nc.tensor.matmul(
                            pV[:, ci * 256 : ci * 256 + 256]
                            if half == 0
                            else pV[:, 0:256],
                            lhsT=xs[:, kp, :, mt * 128 : (mt + 1) * 128],
                            rhs=ws[:, kp, :, c : c + 256],
                            start=(n == 0),
                            stop=(n == 3 * KP - 1),
                            perf_mode=mybir.MatmulPerfMode.DoubleRow,
                        )
                        n += 1
            dv = dsc_sb[:, 2 * KT : 2 * KT + 1]
            if half == 0:
                nc.vector.tensor_scalar_mul(
                    out=v65[:, mt, 0:8, 0:64], in0=pV[:, :cw], scalar1=dv
                )
            else:
                nc.scalar.activation(
                    out=v65[:, mt, 8:12, 0:64],
                    in_=pV[:, 0:cw],
                    func=mybir.ActivationFunctionType.Copy,
                    scale=dv,
                )

        # ---- attention: scores phase ----
        eT_tiles = {}

        def att_sc(h, jts):
            gi, po = h // 2, (h % 2) * 64
            spans = _spans(band[h])
            # first pool rotation (h<6): eT buffers are fresh SBUF, so exp must
            # define the full tile span; afterwards stale-but-finite contents
            # let the ae=0 multiply zero-fill the fine-span gaps.
            fine = spans if h < 6 else _fine_spans(band[h], FINE_L[h])
            if h not in eT_tiles:
                eT_tiles[h] = exp_p.tile([128, JT, 2, S], BF16, tag="eT", name=f"eT{h}")
            eT = eT_tiles[h]
            for jt in jts:
                f0, fw = fine[jt]
                sc = psS.tile([128, 2, S], F32, tag="sc", name=f"sc{h}{jt}")
                for b in range(BPC):
                    t0 = b * S
                    nc.tensor.matmul(
                        sc[:, b, f0 : f0 + fw],
                        lhsT=kq_sb[
                            po : po + DH, 1, gi, t0 + jt * 128 : t0 + (jt + 1) * 128
                        ],
                        rhs=kq_sb[po : po + DH, 0, gi, t0 + f0 : t0 + f0 + fw],
                        start=True,
                        stop=True,
                    )
                nc.scalar.activation(
                    out=eT[:, jt, :, f0 : f0 + fw],
                    in_=sc[:, :, f0 : f0 + fw],
                    func=mybir.ActivationFunctionType.Exp,
                )

        def att_mul(h):
            spans = _spans(band[h])
            if 6 <= h + 2 < H:
                ae_load(h + 2)
            aeh = ae_tiles[h]
            ae0 = ae_offs[h][0]
            eT = eT_tiles[h]
            for jt in range(JT):
                i0, iw = spans[jt]
                off = ae_offs[h][jt] - ae0
                nc.vector.tensor_mul(
                    out=eT[:, jt, :, i0 : i0 + iw],
                    in0=eT[:, jt, :, i0 : i0 + iw],
                    in1=aeh[:, off : off + iw].unsqueeze(1).broadcast_to((128, 2, iw)),
                )

        # ---- attention: per-batch PV + normalize phase ----
        def att_pv(h, b, act_norm=False):
            dt_h = band[h]
            eT = eT_tiles[h]
            pC = psC.tile([128, JT, 128], F32, tag="pC", name=f"pC{h}{b}")
            for it in range(JT):
                jts = [jt for jt in range(JT) if abs(jt - it) <= dt_h]
                for n, jt in enumerate(jts):
                    nc.tensor.matmul(
                        pC[:, it, 0:65],
                        lhsT=eT[:, jt, b, it * 128 : (it + 1) * 128],
                        rhs=v_sb[:, b * JT + jt, h * 65 : h * 65 + 65],
                        start=(n == 0),
                        stop=(n == len(jts) - 1),
                    )
            r = r_p.tile([128, JT], F32, tag="r", name=f"r{h}{b}")
            nc.vector.reciprocal(out=r, in_=pC[:, :, 64:65])
            if act_norm:
                # drain-phase variant: ACT applies the normalize so the final
                # head groups don't serialize on the DVE queue
                for it in range(JT):
                    nc.scalar.activation(
                        out=out_sb[:, b * JT + it, h * DH : (h + 1) * DH],
                        in_=pC[:, it, 0:64],
                        func=mybir.ActivationFunctionType.Copy,
                        scale=r[:, it : it + 1],
                    )
                return
            nc.vector.tensor_mul(
                out=out_sb[:, b * JT : (b + 1) * JT, h * DH : (h + 1) * DH],
                in0=pC[:, :, 0:64],
                in1=r.unsqueeze(2).broadcast_to((128, JT, DH)),
            )

        def out_chunk(c0, cw):
            # column chunk across all token tiles: row (mt,p), cols c0:c0+cw
            nc.sync.dma_start(
                out=out.rearrange("(mt p) d -> p mt d", p=128)[:, :, c0 : c0 + cw],
                in_=out_sb[:, :, c0 : c0 + cw],
            )

        # ---- schedule: strict rounds. Per round r: project ft-tile r (its
        # psum-freeing copies land at the HEAD of the DVE/ACT queues, before
        # this round's muls), score heads 2r/2r+1 with the two heads' jt
        # tiles interleaved (hides exp latency behind the other head's
        # matmuls), multiply, then PV heads from two rounds back. ----
        def sc_pair(a):
            att_sc(a, (0, 1))
            att_sc(a + 1, (0, 1))
            att_sc(a, (2, 3))
            att_sc(a + 1, (2, 3))

        # rounds: projections run one ft-tile ahead of the scores that
        # consume them, so sc-matmuls never wait on same-round copies
        proj_qk(0, 0)
        proj_qk(1, 0, pool=psS)
        proj_qk(0, 1)
        proj_qk(1, 1, pool=psS)
        sc_pair(0)
        att_mul(0)
        att_mul(1)
        # r1
        proj_qk(0, 2, pool=psC)
        proj_qk(1, 2, pool=psC)
        sc_pair(2)
        for mt in range(4):
            proj_v(mt, 0)
        att_mul(2)
        att_mul(3)
        # r2
        proj_qk(0, 3)
        proj_qk(1, 3)
        sc_pair(4)
        for mt in range(4, MT):
            proj_v(mt, 0)
        att_mul(4)
        att_mul(5)
        att_pv(0, 0)
        att_pv(1, 0)
        att_pv(0, 1)
        att_pv(1, 1)
        # r3
        proj_qk(0, 4)
        proj_qk(1, 4)
        sc_pair(6)
        att_mul(6)
        att_mul(7)
        att_pv(2, 0)
        att_pv(2, 1)
        att_pv(3, 0)
        att_pv(3, 1)
        out_chunk(0, 256)
        # r4
        proj_qk(0, 5)
        proj_qk(1, 5)
        sc_pair(8)
        for mt in range(4):
            proj_v(mt, 1)
        att_mul(8)
        att_mul(9)
        att_pv(4, 0)
        att_pv(4, 1)
        att_pv(5, 0)
        att_pv(5, 1)
        # r5
        sc_pair(10)
        for mt in range(4, MT):
            proj_v(mt, 1)
        att_mul(10)
        att_mul(11)
        att_pv(6, 0)
        att_pv(6, 1)
        att_pv(7, 0)
        att_pv(7, 1)
        out_chunk(256, 256)
        att_pv(8, 0)
        att_pv(8, 1)
        att_pv(9, 0)
        att_pv(9, 1)
        att_pv(10, 0)
        att_pv(11, 0)
        nc.sync.dma_start(
            out=out.rearrange("(mt p) d -> p mt d", p=128)[:, 0:4, 512:768],
            in_=out_sb[:, 0:4, 512:768],
        )
        # r6
        att_pv(10, 1)
        att_pv(11, 1)
        nc.sync.dma_start(
            out=out.rearrange("(mt p) d -> p mt d", p=128)[:, 4:8, 512:768],
            in_=out_sb[:, 4:8, 512:768],
        )
    _split_multi_waits(nc)
    return nc


def host_prep(inputs: dict, band=None):
    """Returns (shared input dict, per-core dict list)."""
    import ml_dtypes

    band = band or BAND
    ae_offs, ae_total = _ae_layout(band)

    hs = np.ascontiguousarray(np.asarray(inputs["hidden_states"], np.float32))
    Wq = np.asarray(inputs["Wq"], np.float32)
    Wk = np.asarray(inputs["Wk"], np.float32)
    Wv = np.asarray(inputs["Wv"], np.float32)
    qfc = np.asarray(inputs["query_fc"], np.float32)
    kfc = np.asarray(inputs["key_fc"], np.float32)
    mwt = np.asarray(inputs["mixture_weight"], np.float32)[0, :, 0, 0, :]  # [H,2]

    e = np.exp(mwt - mwt.max(-1, keepdims=True))
    mw = e / e.sum(-1, keepdims=True)
    qscale = np.repeat(mw[:, 0] / np.sqrt(DH), DH).astype(np.float32)  # per out col

    ax = FP8_MAX / max(np.abs(hs).max(), 1e-30)

    def quant_w(wT):  # wT [in_feat, out_feat] -> (fp8 array, descale per col)
        amax = np.abs(wT).max(axis=0)
        ac = FP8_MAX / np.maximum(amax, 1e-30)
        w8 = (wT * ac[None, :]).astype(ml_dtypes.float8_e4m3)
        return w8, (1.0 / (ax * ac)).astype(np.float32)

    def quant_w_rails(wT):
        w8, d = quant_w(wT)
        ac = FP8_MAX / np.maximum(np.abs(wT).max(axis=0), 1e-30)
        w8l = ((wT * ac[None, :]) - w8.astype(np.float32)).astype(
            ml_dtypes.float8_e4m3
        )
        return w8, w8l, d

    wq8, wq8l, dq = quant_w_rails(Wq.T * qscale[None, :])
    wk8, wk8l, dk = quant_w_rails(Wk.T)

    # dual-rail fp8 V weights at one global scale
    wvT = Wv.T.astype(np.float32)
    aw = FP8_MAX / max(np.abs(wvT).max(), 1e-30)
    wv8 = (wvT * aw).astype(ml_dtypes.float8_e4m3)
    wv8l = ((wvT - wv8.astype(np.float32) / aw) * aw).astype(ml_dtypes.float8_e4m3)
    dv = np.full((128, 1), 1.0 / (ax * aw), np.float32)

    dsc = np.concatenate(
        [dq.reshape(KT, 128).T, dk.reshape(KT, 128).T, dv], axis=1
    )  # [128, 2*KT+1]
    dsc = np.ascontiguousarray(dsc)

    # packed banded bias table: ae[h][p, off+e] = exp(mw1*synth^T + alibi)
    synthT = np.einsum("hik,hjk->hji", qfc, kfc).astype(np.float32)
    pos = np.arange(S)
    absd = np.abs(pos[None, :] - pos[:, None]).astype(np.float32)
    slopes = SLOPES.astype(np.float32)
    bias = mw[:, 1][:, None, None] * synthT - slopes[:, None, None] * absd[None]
    aexp = np.exp(bias)  # [H, S(j), S(i)]
    ae_pack = np.zeros((128, ae_total), np.float32)
    for h in range(H):
        sp = _spans(band[h])
        fine = _fine_spans(band[h], FINE_L[h])
        for jt in range(JT):
            i0, iw = sp[jt]
            blk = aexp[h, jt * 128 : (jt + 1) * 128, i0 : i0 + iw].copy()
            f0, fw = fine[jt]
            blk[:, : f0 - i0] = 0.0
            blk[:, f0 - i0 + fw :] = 0.0
            ae_pack[:, ae_offs[h][jt] : ae_offs[h][jt] + iw] = blk
    ae_pack = np.ascontiguousarray(ae_pack.astype(ml_dtypes.bfloat16))

    shared = dict(
        wq8=wq8, wq8l=wq8l, wk8=wk8, wk8l=wk8l, wv8=wv8, wv8l=wv8l, dsc=dsc, ae=ae_pack
    )
    n_cores = hs.shape[0] // BPC
    per_core = []
    for c in range(n_cores):
        xT = np.ascontiguousarray(hs[c * BPC : (c + 1) * BPC].reshape(T, D).T)
        x8 = (xT * ax).astype(ml_dtypes.float8_e4m3)
        x8l = ((xT - x8.astype(np.float32) / ax) * ax).astype(ml_dtypes.float8_e4m3)
        per_core.append(
            dict(x8=np.ascontiguousarray(x8), x8l=np.ascontiguousarray(x8l))
        )
    return shared, per_core


# ---------------------------------------------------------------------------
# Harness entry point: full (unsharded) inputs -> full output.
# Shards batch 16 -> 8 cores x 2, runs the SPMD Bass kernel, gathers.
# ---------------------------------------------------------------------------

N_CORES = 8
_NC_CACHE: dict = {}


def kernel(**inputs) -> np.ndarray:
    shared, per_core = host_prep(inputs)
    if "nc" not in _NC_CACHE:
        _NC_CACHE["nc"] = build_nc()
    nc = _NC_CACHE["nc"]
    in_maps = [dict(shared, **per_core[c]) for c in range(N_CORES)]
    from concourse.bass_utils import run_bass_kernel_spmd

    res = run_bass_kernel_spmd(nc, in_maps, core_ids=list(range(N_CORES)))
    outs = [
        res.results[c]["out"].astype(np.float32).reshape(BPC, S, D)
        for c in range(N_CORES)
    ]
    return np.concatenate(outs, axis=0)
